# revision 31
# baseline (speedup 1.0000x reference)
"""Trainium2 Bass kernel for CustomizablePatchDominantGradientOrientation.

Pipeline per patch (32x32, fp32):
  sobel (replicate pad, [1,2,1]x[-1,0,1] separable; /8 dropped - the final
  angle is invariant to a global scale on (gx, gy, mag))
  mag = sqrt(gx^2+gy^2+eps'), theta = 2*atan(gy/(mag+gx))  (half-angle atan2)
  soft 36-bin histogram of theta weighted by mag, via the relu-knot
  decomposition: with U = t*M (t = angle in bin units), M = mag,
    R_j = sum relu(U - j*M),  L_j = sum relu(j*M - U)
    hist[k] = second difference of L (bins 0..18) / R (bins 19..35),
    wrap bin 0 += R_17.
  The knot passes run as custom DVE ops on fp16-packed U/M at 2 elem/cycle
  (hand-written 2x_1p uop programs; engine falls back to the 1x program if
  the mem-pattern doesn't qualify).
  circular [w0,w1,w2] smoothing, argmax, parabolic refinement -> angle.

Data parallel: B=32768 patches sharded over 8 NeuronCores (4096 each);
per core 32 tiles of [128 patches x 1024 pixels].  Layout is patch-major:
partitions = patches, free axis = pixels.
"""

import math

import numpy as np

NBINS = 36
PI = math.pi
PATCH = 32
HW = PATCH * PATCH
P = 128          # partitions (patches per tile)
N_CORES = 8
GROUP = 4        # tiles per ACT-table-set phase group
GPSIMD_OFFLOAD = True  # host big contiguous sobel TTs on the Pool engine
_DEBUG_DUMP = False    # add dbg_rl / dbg_hs outputs

_BUILD_CACHE = {}
_OPS_REGISTERED = {}


# --------------------------------------------------------------------------
# custom DVE ops
# --------------------------------------------------------------------------
def _register_custom_ops():
    """Register the custom DVE ops at runtime (row assignment + sha pin,
    exactly what a source-level `OPS.append` would do)."""
    if _OPS_REGISTERED:
        return _OPS_REGISTERED
    from operator import add as _op_add

    import concourse.dve_ops as dve_ops
    from concourse.dve_ops import DveOp, _COMPILE_CACHE
    from concourse.dve_spec import (
        Spec, Src0, Src1, C0, C1, Zero, relu, maxx, lower, _has_src1, sq,
    )
    from concourse.dve_uop import (
        DveOpSpec, UopConfig, UopDpConfig, AluOp, AluInp, DelayInp, InpSel,
        OutSel, OutPath, Trigger, ENABLE, DISABLE, _MAX_LANES,
    )

    # ---- dual-knot uop programs (fp32, 1x) -------------------------------
    # One pass accumulates TWO relu-knot sums:
    #   accA (stage-6 a_flop) = sum relu(U - c0*M)        [R] / relu(c0*M - U) [L]
    #   accB (stage-7 a_flop) = sum relu(U - (c0+c1)*M)   [R] / ...           [L]
    # accB is read by the auto-emitted DVE_READ_ACCUMULATOR2_ANT (accum_out);
    # accA by the KNOT_RDA op below (mimics stock DVE_READ_ACCUMULATOR's
    # program, which reads the stage-6 a_flop).
    # Input slots: 1:SRC_0(U) 2:SRC_1(M) 3:CONST_0(c0) 4:CONST_1(step) 5:ZERO
    # -> lanes 0:U 1:M 2:c0 3:step 4:ZERO; lane5 parks d0.
    def _dual_dp(left, seed):
        Pd = DelayInp.PREV_DELAY
        Ad = DelayInp.PREV_ALU_OUT
        I = AluInp

        def blk(op, s0, s1, park=None, a_out=False):
            d = [Pd] * 6 + [Ad] * (_MAX_LANES - 6)
            if park is not None:
                d[park] = Ad
            c = UopDpConfig(op=op, alu_src0=s0, alu_src1=s1,
                            delay=d,
                            delay_enable=[ENABLE] * 6 + [DISABLE] * (_MAX_LANES - 6),
                            alu_out_enable=ENABLE)
            if a_out:
                c.alu_out_a_enable = ENABLE
            return c

        sub0 = ((I.PREV_DELAY_0, I.PREV_ALU_OUT) if not left
                else (I.PREV_ALU_OUT, I.PREV_DELAY_0))
        # d1 = d0 - q (R) / d0 + q (L)
        d1op, d1s = ((AluOp.SUBTRACT, (I.PREV_DELAY_5, I.PREV_ALU_OUT))
                     if not left else
                     (AluOp.ADD, (I.PREV_ALU_OUT, I.PREV_DELAY_5)))
        dps = [
            blk(AluOp.MULTIPLY, I.PREV_DELAY_2, I.PREV_DELAY_1),           # p = c0*M
            blk(AluOp.SUBTRACT, *sub0),                                    # d0
            blk(AluOp.MULTIPLY, I.PREV_DELAY_3, I.PREV_DELAY_1, park=5),   # q = step*M; park d0
            blk(d1op, *d1s),                                               # d1
            blk(AluOp.MAX, I.PREV_DELAY_5, I.PREV_DELAY_4, park=0),        # r0 = relu(d0); park d1
            blk(AluOp.MAX, I.PREV_DELAY_0, I.PREV_DELAY_4, park=1),        # r1 = relu(d1); park r0
            blk(AluOp.ADD, I.CURR_ALU_OUT, I.PREV_DELAY_1, park=2,
                a_out=True),                                               # accA += r0; park r1
            blk(AluOp.ADD, I.CURR_ALU_OUT, I.PREV_DELAY_2, a_out=True),    # accB += r1
        ]
        if seed:
            for st in (6, 7):
                dps[st].op = AluOp.BYPASS
                dps[st].alu_src0 = AluInp.PREV_DELAY_4
                dps[st].alu_src1 = AluInp.PREV_DELAY_4
        return dps

    def _dual_uops(left):
        """5-state program: seed -> steady (accumulate) -> 4-cycle drain pad
        -> emit accA -> emit accB.  The two accumulator values are the
        instruction's own 2-element dst stream, so no separate accumulator
        read instructions are needed and no engine state must survive across
        instructions (the scheduler may interleave anything)."""
        inp = [InpSel.ZERO] * 8
        inp_en = [DISABLE] * 8
        for slot, sel in ((1, InpSel.SRC_0), (2, InpSel.SRC_1),
                          (3, InpSel.CONST_0), (4, InpSel.CONST_1),
                          (5, InpSel.ZERO)):
            inp[slot] = sel
            inp_en[slot] = ENABLE
        seed = UopConfig(
            inp=list(inp), inp_enable=list(inp_en),
            trigger=(Trigger.COUNT, Trigger.NONE, Trigger.NONE),
            next_uop=(1, 0, 0), repeat_count=1, accum_enabled=ENABLE,
            datapath_config=_dual_dp(left, seed=True))
        steady = UopConfig(
            inp=list(inp), inp_enable=list(inp_en),
            trigger=(Trigger.SRC_TENSOR_DONE, Trigger.NONE, Trigger.NONE),
            next_uop=(2, 0, 0), repeat_count=0,
            require_inp0=ENABLE, require_inp1=ENABLE, accum_enabled=ENABLE,
            datapath_config=_dual_dp(left, seed=False))

        def _bypass_dps(read_stage=None):
            dps = []
            for st in range(8):
                c = UopDpConfig(op=AluOp.BYPASS,
                                alu_src0=AluInp.PREV_ALU_OUT,
                                alu_src1=AluInp.PREV_ALU_OUT,
                                delay=[DelayInp.PREV_DELAY] * _MAX_LANES,
                                delay_enable=[DISABLE] * _MAX_LANES,
                                alu_out_enable=ENABLE)
                dps.append(c)
            if read_stage is not None:
                dps[read_stage].alu_src0 = AluInp.NEXT_ALU_OUT_A
                dps[read_stage].alu_src1 = AluInp.NEXT_ALU_OUT_A
            return dps

        pad = UopConfig(
            inp=list(inp), inp_enable=list(inp_en),
            trigger=(Trigger.COUNT, Trigger.NONE, Trigger.NONE),
            next_uop=(3, 0, 0), repeat_count=4, accum_enabled=ENABLE,
            datapath_config=_bypass_dps())
        fin_a = UopConfig(
            inp=list(inp), inp_enable=list(inp_en),
            trigger=(Trigger.COUNT, Trigger.NONE, Trigger.NONE),
            next_uop=(4, 0, 0), repeat_count=1, accum_enabled=ENABLE,
            datapath_config=_bypass_dps(read_stage=5))
        fin_a.enable_output(OutSel.ALU_OUT, OutPath.WR0_LO)
        fin_b = UopConfig(
            inp=list(inp), inp_enable=list(inp_en),
            trigger=(Trigger.COUNT, Trigger.NONE, Trigger.NONE),
            next_uop=(0, 0, 0), repeat_count=1, accum_enabled=ENABLE,
            datapath_config=_bypass_dps(read_stage=6))
        fin_b.enable_output(OutSel.ALU_OUT, OutPath.WR0_LO)
        return [seed, steady, pad, fin_a, fin_b]

    def _rd2_uops():
        """Read BOTH a_flops in one 1-cycle op and write them as two
        consecutive dst elements: dst[0] = accA (stage-6 a_flop, via WR0_LO),
        dst[1] = accB (stage-7 a_flop, via WR1_LO)."""
        Pd = DelayInp.PREV_DELAY
        I = AluInp
        dps = []
        for st in range(8):
            c = UopDpConfig(op=AluOp.BYPASS,
                            alu_src0=I.PREV_ALU_OUT, alu_src1=I.PREV_ALU_OUT,
                            delay=[Pd] * _MAX_LANES,
                            delay_enable=[DISABLE] * _MAX_LANES,
                            alu_out_enable=ENABLE)
            dps.append(c)
        dps[5].alu_src0 = I.NEXT_ALU_OUT_A       # accA (stage-6 a_flop)
        dps[5].alu_src1 = I.NEXT_ALU_OUT_A
        dps[6].alu_src0 = I.NEXT_ALU_OUT_A       # accB (stage-7 a_flop)
        dps[6].alu_src1 = I.NEXT_ALU_OUT_A
        dps[6].delay = [DelayInp.PREV_ALU_OUT] + [Pd] * (_MAX_LANES - 1)
        dps[6].delay_enable = [ENABLE] + [DISABLE] * (_MAX_LANES - 1)
        dps[7].delay_enable = [ENABLE] + [DISABLE] * (_MAX_LANES - 1)
        u = UopConfig(
            inp=[InpSel.ZERO] * 8,
            inp_enable=[DISABLE] * 8,
            trigger=(Trigger.COUNT, Trigger.NONE, Trigger.NONE),
            next_uop=(0, 0, 0), repeat_count=1,
            require_inp0=ENABLE,  # consume the 1-elem in0 (no rd-FIFO leak)
            datapath_config=dps)
        u.enable_input(InpSel.SRC_0, 1)
        u.enable_output(OutSel.DELAY_0, OutPath.WR0_LO)   # accA
        u.enable_output(OutSel.ALU_OUT, OutPath.WR1_LO)   # accB
        return [u]

    def _rda_uops():
        """Read the stage-6 a_flop (accA) and write it as a [P,1] scalar —
        the stock DVE_READ_ACCUMULATOR program (opcode-table slot 117)
        rebuilt as a custom row: one COUNT cycle, stage-5 ALU passes
        NEXT_ALU_OUT_A (= stage-6 a_flop), bypass chain to the writer."""
        Pd = DelayInp.PREV_DELAY
        I = AluInp
        dps = []
        for st in range(8):
            c = UopDpConfig(op=AluOp.BYPASS,
                            alu_src0=I.PREV_ALU_OUT, alu_src1=I.PREV_ALU_OUT,
                            delay=[Pd] * 6 + [DelayInp.PREV_ALU_OUT] * (_MAX_LANES - 6),
                            delay_enable=[DISABLE] * _MAX_LANES,
                            alu_out_enable=ENABLE)
            dps.append(c)
        dps[5].alu_src0 = I.NEXT_ALU_OUT_A
        dps[5].alu_src1 = I.NEXT_ALU_OUT_A
        u = UopConfig(
            inp=[InpSel.ZERO] * 8,
            inp_enable=[DISABLE] * 8,
            trigger=(Trigger.COUNT, Trigger.NONE, Trigger.NONE),
            next_uop=(0, 0, 0), repeat_count=1,
            require_inp0=ENABLE,  # consume the 1-elem in0 (no rd-FIFO leak)
            datapath_config=dps)
        u.enable_input(InpSel.SRC_0, 1)
        u.enable_output(OutSel.ALU_OUT, OutPath.WR0_LO)
        return [u]

    def _ref_dual_r(in0, in1, s0, s1, imm2):
        b = np.maximum(in0 - s0 * in1, 0.0).astype(np.float32)
        b2 = np.maximum(in0 - (s0 + s1) * in1, 0.0).astype(np.float32)
        return b, b2.reshape(b2.shape[0], -1).sum(axis=-1, keepdims=True)

    def _ref_dual_l(in0, in1, s0, s1, imm2):
        b = np.maximum(s0 * in1 - in0, 0.0).astype(np.float32)
        b2 = np.maximum((s0 + s1) * in1 - in0, 0.0).astype(np.float32)
        return b, b2.reshape(b2.shape[0], -1).sum(axis=-1, keepdims=True)

    def _ref_rda(in0, in1, s0, s1, imm2):
        return in0.astype(np.float32)

    def _rsqrt_nr_ref(in0, in1, s0, s1, imm2):
        return ((s0 - in0 * in1 * in1 * s1) * in1).astype(np.float32)

    def _addmax_ref(in0, in1, s0, s1, imm2):
        return np.maximum(in0 + in1, s0).astype(np.float32)

    def _reg(name, spec, uops=None, rd1_en=None):
        if name in dve_ops._SUB_OPCODE_FOR_NAME:
            for op in dve_ops.OPS:
                if op.name == name:
                    return op
        row = dve_ops._CUSTOM_DVE_ROW_BASE + len(dve_ops.OPS)
        assert row < 0x20, "custom-DVE row budget exhausted"
        dve_ops._SUB_OPCODE_FOR_NAME[name] = row
        shas = {}
        for ver in ("v3", "v4"):
            s = DveOpSpec(name=name, opcode=row,
                          uops=(uops if uops is not None
                                else lower(spec, ver=ver)),
                          rd1_en=(rd1_en if rd1_en is not None
                                  else _has_src1(spec)))
            s.validate(ver)
            shas[ver] = s.sha(ver)
            _COMPILE_CACHE[(name, ver)] = s
        op = DveOp(name, spec, subdim=False, uops_sha=shas)
        dve_ops.OPS.append(op)
        dve_ops.CUSTOM_DVE_SPECS[name] = spec
        return op

    _OPS_REGISTERED["knot2_r"] = _reg(
        "KNOT2_R_ANT",
        Spec(body=relu(Src0 - C0 * Src1), accum=_op_add, accum_init=Zero,
             reference=_ref_dual_r),
        uops=_dual_uops(False), rd1_en=True)
    _OPS_REGISTERED["knot2_l"] = _reg(
        "KNOT2_L_ANT",
        Spec(body=relu(C0 * Src1 - Src0), accum=_op_add, accum_init=Zero,
             reference=_ref_dual_l),
        uops=_dual_uops(True), rd1_en=True)
    _OPS_REGISTERED["rda"] = _reg(
        "KNOT_RDA_ANT",
        Spec(body=Src0 * C0, reference=_ref_rda),
        uops=_rda_uops(), rd1_en=False)
    _OPS_REGISTERED["rd2"] = _reg(
        "KNOT_RD2_ANT",
        Spec(body=Src0 * C0, reference=_ref_rda),
        uops=_rd2_uops(), rd1_en=False)
    # z1 = (c0 - g2*z0^2*c1)*z0  (one Newton step toward 1/sqrt(g2))
    _OPS_REGISTERED["rsqrt_nr"] = _reg(
        "RSQRT_NR_ANT",
        Spec(body=(C0 - Src0 * sq(Src1) * C1) * Src1,
             reference=_rsqrt_nr_ref))
    _OPS_REGISTERED["addmax"] = _reg(
        "ADD_MAX_ANT",
        Spec(body=maxx(Src0 + Src1, C0), reference=_addmax_ref))
    return _OPS_REGISTERED


# --------------------------------------------------------------------------
# kernel build
# --------------------------------------------------------------------------
def _build(b_core, smooth_w, wk_is_ones):
    import concourse.bacc as bacc
    import concourse.mybir as mybir
    from concourse.tile import TileContext
    from concourse.bass import broadcast_tensor_aps

    ops = _register_custom_ops()
    KNOT2_R, KNOT2_L, RDA = ops["knot2_r"], ops["knot2_l"], ops["rda"]
    RSQRT_NR, ADDMAX = ops["rsqrt_nr"], ops["addmax"]
    # NEFF-cache salt: the uop tables are side data the compile cache does not
    # key on; bake their hash into an unused immediate so table edits recompile.
    salt = float(sum(int(op.uops_sha["v3"], 16) for op in ops.values())
                 % 1000003) / 1000.0

    f32 = mybir.dt.float32
    f16 = mybir.dt.float16
    Alu = mybir.AluOpType
    Act = mybir.ActivationFunctionType

    n_tiles = b_core // P
    assert b_core % P == 0
    w0, w1, w2 = (float(x) for x in smooth_w)

    nc = bacc.Bacc(None, target_bir_lowering=False, debug=False)
    patch_in = nc.dram_tensor("patch", [b_core, HW], f32, kind="ExternalInput")
    # consts: iota36 repeated n_tiles times, then (iota36 - 64) repeated
    consts_in = nc.dram_tensor("consts", [P, 2 * n_tiles * NBINS], f32,
                               kind="ExternalInput")
    wk_in = None
    if not wk_is_ones:
        wk_in = nc.dram_tensor("wk", [P, HW], f32, kind="ExternalInput")
    out_t = nc.dram_tensor("angle", [b_core], f32, kind="ExternalOutput")
    dbg = {}
    if _DEBUG_DUMP:
        dbg["rl"] = nc.dram_tensor("dbg_rl", [P, n_tiles * 41], f32,
                                   kind="ExternalOutput")
        dbg["hs"] = nc.dram_tensor("dbg_hs", [P, n_tiles * NBINS], f32,
                                   kind="ExternalOutput")

    with TileContext(nc) as tc:
        with tc.tile_pool(name="pool", bufs=2) as pool, \
             tc.tile_pool(name="persist", bufs=1) as pp:
            IOTA = pp.tile([P, n_tiles, NBINS], f32)
            IOTA64 = pp.tile([P, n_tiles, NBINS], f32)
            nc.sync.dma_start(IOTA[:], consts_in[:, 0:n_tiles * NBINS])
            nc.sync.dma_start(IOTA64[:], consts_in[:, n_tiles * NBINS:])
            WK = None
            if wk_in is not None:
                WK = pp.tile([P, HW], f32)
                nc.sync.dma_start(WK[:], wk_in[:])

            # knot sums: L_j (j=-19..1) at 0..20, R_j (j=0..18) at 21..39,
            # slot 40 = discard (unused second knot of the last L pair).
            # L_-19 = L_-18 = 0 (t > -18 always) and R_18 = 0 (t <= 18):
            # those slots stay at the memset value and their passes are skipped.
            RL = pp.tile([P, n_tiles, 41], f32)
            nc.vector.memset(RL[:], 0.0)
            HISTE = pp.tile([P, n_tiles, NBINS + 2], f32)
            ANG = pp.tile([P, n_tiles], f32)

            n_groups = (n_tiles + GROUP - 1) // GROUP
            for g in range(n_groups):
                tiles = range(g * GROUP, min((g + 1) * GROUP, n_tiles))
                slot = {}
                # ---- phase A: sobel, magnitude (sqrt table set) ----
                for t in tiles:
                    s = t % GROUP
                    X = pool.tile([P, HW], f32, tag="x", bufs=3, name=f"x{t}")
                    nc.sync.dma_start(X[:], patch_in[t * P:(t + 1) * P, :])
                    X3 = X.rearrange("p (r c) -> p r c", c=PATCH)

                    SV = pool.tile([P, HW], f32, tag="sv", name=f"sv{t}")
                    # vertical [1,2,1] with replicate rows
                    nc.vector.scalar_tensor_tensor(
                        out=SV[:, 32:992], in0=X[:, 32:992], scalar=2.0,
                        in1=X[:, 0:960], op0=Alu.mult, op1=Alu.add)
                    sv_eng = nc.gpsimd if GPSIMD_OFFLOAD else nc.vector
                    sv_eng.tensor_tensor(
                        SV[:, 32:992], SV[:, 32:992], X[:, 64:1024], Alu.add)
                    nc.vector.scalar_tensor_tensor(
                        out=SV[:, 0:32], in0=X[:, 0:32], scalar=3.0,
                        in1=X[:, 32:64], op0=Alu.mult, op1=Alu.add)
                    nc.vector.scalar_tensor_tensor(
                        out=SV[:, 992:1024], in0=X[:, 992:1024], scalar=3.0,
                        in1=X[:, 960:992], op0=Alu.mult, op1=Alu.add)
                    SV3 = SV.rearrange("p (r c) -> p r c", c=PATCH)

                    GX = pool.tile([P, HW], f32, tag=f"gx{s}", bufs=1,
                                   name=f"gx{t}")
                    GX3 = GX.rearrange("p (r c) -> p r c", c=PATCH)
                    # horizontal central difference with replicate cols
                    nc.vector.tensor_tensor(
                        GX3[:, :, 1:31], SV3[:, :, 2:32], SV3[:, :, 0:30],
                        Alu.subtract)
                    nc.vector.tensor_tensor(
                        GX3[:, :, 0:1], SV3[:, :, 1:2], SV3[:, :, 0:1],
                        Alu.subtract)
                    nc.vector.tensor_tensor(
                        GX3[:, :, 31:32], SV3[:, :, 31:32], SV3[:, :, 30:31],
                        Alu.subtract)

                    SH = pool.tile([P, HW], f32, tag="sh", name=f"sh{t}")
                    SH3 = SH.rearrange("p (r c) -> p r c", c=PATCH)
                    # horizontal [1,2,1] with replicate cols
                    nc.vector.scalar_tensor_tensor(
                        out=SH3[:, :, 1:31], in0=X3[:, :, 1:31], scalar=2.0,
                        in1=X3[:, :, 0:30], op0=Alu.mult, op1=Alu.add)
                    nc.vector.tensor_tensor(
                        SH3[:, :, 1:31], SH3[:, :, 1:31], X3[:, :, 2:32],
                        Alu.add)
                    nc.vector.scalar_tensor_tensor(
                        out=SH3[:, :, 0:1], in0=X3[:, :, 0:1], scalar=3.0,
                        in1=X3[:, :, 1:2], op0=Alu.mult, op1=Alu.add)
                    nc.vector.scalar_tensor_tensor(
                        out=SH3[:, :, 31:32], in0=X3[:, :, 31:32], scalar=3.0,
                        in1=X3[:, :, 30:31], op0=Alu.mult, op1=Alu.add)

                    GY = pool.tile([P, HW], f32, tag=f"gy{s}", bufs=1,
                                   name=f"gy{t}")
                    # vertical central difference with replicate rows
                    gy_eng = nc.gpsimd if GPSIMD_OFFLOAD else nc.vector
                    gy_eng.tensor_tensor(
                        GY[:, 32:992], SH[:, 64:1024], SH[:, 0:960],
                        Alu.subtract)
                    nc.vector.tensor_tensor(
                        GY[:, 0:32], SH[:, 32:64], SH[:, 0:32], Alu.subtract)
                    nc.vector.tensor_tensor(
                        GY[:, 992:1024], SH[:, 992:1024], SH[:, 960:992],
                        Alu.subtract)

                    if WK is not None:
                        nc.vector.tensor_tensor(GX[:], GX[:], WK[:], Alu.mult)
                        nc.vector.tensor_tensor(GY[:], GY[:], WK[:], Alu.mult)

                    # g2 = gx^2 + gy^2 + eps  (eps scaled by 8^2 vs reference)
                    # sv/sh slots are dead here; reuse their tags for squares.
                    # Exact fp32 multiplies on GPSIMD (ACT Square is ~1e-5
                    # off, which poisons the magnitude beyond repair).
                    X2 = pool.tile([P, HW], f32, tag="sv", name=f"x2{t}")
                    Y2 = pool.tile([P, HW], f32, tag="sh", name=f"y2{t}")
                    nc.gpsimd.tensor_tensor(X2[:], GX[:], GX[:], Alu.mult)
                    nc.gpsimd.tensor_tensor(Y2[:], GY[:], GY[:], Alu.mult)
                    G2 = pool.tile([P, HW], f32, tag="g2", name=f"g2{t}")
                    nc.vector.scalar_tensor_tensor(
                        out=G2[:], in0=X2[:], scalar=6.4e-17, in1=Y2[:],
                        op0=Alu.add, op1=Alu.add)
                    M = pool.tile([P, HW], f32, tag=f"m{s}", bufs=1,
                                   name=f"m{t}")
                    nc.scalar.activation(M[:], G2[:], Act.Sqrt)
                    # one Newton step: m = g2 * nr(1/m0)
                    RC = pool.tile([P, HW], f32, tag="rc", name=f"rc{t}")
                    SC = pool.tile([P, HW], f32, tag="sc", name=f"sc{t}")
                    nc.vector.reciprocal_approx_fast(RC[:], M[:])
                    nc.vector._custom_dve(RSQRT_NR, out=SC[:], in0=G2[:],
                                          in1=RC[:], s0=1.5, s1=0.5)
                    nc.vector.tensor_tensor(M[:], G2[:], SC[:], Alu.mult)
                    slot[t] = (GX, GY, M)

                # ---- phase B: orientation + knot histogram (arctan set) --
                for t in tiles:
                    GX, GY, M = slot[t]
                    # d = max(m + gx, 1e-30): the clamp both avoids the
                    # recip(0)=NaN edge and pins rounding-negative d to the
                    # correct wrap side (t -> 36/0 by sign of gy).
                    D = pool.tile([P, HW], f32, tag="g2", name=f"d{t}")
                    nc.vector._custom_dve(ADDMAX, out=D[:], in0=M[:],
                                          in1=GX[:], s0=1e-30)
                    RC = pool.tile([P, HW], f32, tag="rc", name=f"rcb{t}")
                    SC = pool.tile([P, HW], f32, tag="sc", name=f"scb{t}")
                    nc.vector.reciprocal_approx_accurate(RC[:], D[:], SC[:])
                    nc.vector.tensor_tensor(SC[:], GY[:], RC[:], Alu.mult)
                    A = pool.tile([P, HW], f32, tag="a", name=f"a{t}")
                    nc.scalar.activation(A[:], SC[:], Act.Arctan)

                    # U = A*M (atan units premultiplied by M); knots j*pi/36.
                    U = pool.tile([P, HW], f32, tag="u", name=f"u{t}")
                    nc.gpsimd.tensor_tensor(U[:], A[:], M[:], Alu.mult)

                    hb = PI / 36.0  # knot spacing in atan units

                    def dual(op, j0, slotA):
                        # writes (accA, accB) = (knot j0, knot j0+1) as the
                        # instruction's own 2-element dst stream
                        nc.vector._custom_dve(
                            op, out=RL[:, t, slotA:slotA + 2],
                            in0=U[:], in1=M[:],
                            s0=float(j0) * hb, s1=hb, imm2=salt)

                    # L pairs: knots (j, j+1) -> slots (j+19, j+20); the last
                    # pair's L_2 lands on slot 21, overwritten by R_0 below.
                    for j in range(-17, 2, 2):
                        dual(KNOT2_L, j, j + 19)
                    for j in range(0, 18, 2):     # R pairs -> slots 21+j, 22+j
                        dual(KNOT2_R, j, 21 + j)

            # ---- tail: D2, smoothing, argmax, refinement (batched) ----
            # bins 0..18 from L: hist[k] = L[k] - 2 L[k+1] + L[k+2]
            # bins 19..35 from R: hist[k] = R[k-19] - 2 R[k-18] + R[k-17]
            # wrap: hist[0] += R_17
            HC = HISTE[:, :, 1:37]  # core 36 bins
            T1 = pp.tile([P, n_tiles, 19], f32)
            nc.vector.scalar_tensor_tensor(
                out=T1[:], in0=RL[:, :, 1:20], scalar=-2.0,
                in1=RL[:, :, 0:19], op0=Alu.mult, op1=Alu.add)
            nc.vector.tensor_tensor(HC[:, :, 0:19], T1[:], RL[:, :, 2:21],
                                    Alu.add)
            T2 = pp.tile([P, n_tiles, 17], f32)
            nc.vector.scalar_tensor_tensor(
                out=T2[:], in0=RL[:, :, 22:39], scalar=-2.0,
                in1=RL[:, :, 21:38], op0=Alu.mult, op1=Alu.add)
            nc.vector.tensor_tensor(HC[:, :, 19:36], T2[:], RL[:, :, 23:40],
                                    Alu.add)
            nc.vector.tensor_tensor(HC[:, :, 0:1], HC[:, :, 0:1],
                                    RL[:, :, 38:39], Alu.add)

            # wrap columns for circular smoothing
            nc.vector.tensor_copy(HISTE[:, :, 0:1], HISTE[:, :, 36:37])
            nc.vector.tensor_copy(HISTE[:, :, 37:38], HISTE[:, :, 1:2])

            SM = pp.tile([P, n_tiles, NBINS], f32)
            nc.vector.tensor_scalar(SM[:], HISTE[:, :, 2:38], w2, None,
                                    Alu.mult)
            nc.vector.scalar_tensor_tensor(
                out=SM[:], in0=HISTE[:, :, 0:36], scalar=w0, in1=SM[:],
                op0=Alu.mult, op1=Alu.add)
            HS = pp.tile([P, n_tiles, NBINS], f32)
            nc.vector.scalar_tensor_tensor(
                out=HS[:], in0=HISTE[:, :, 1:37], scalar=w1, in1=SM[:],
                op0=Alu.mult, op1=Alu.add)

            VMAX = pp.tile([P, n_tiles, 1], f32)
            nc.vector.tensor_reduce(VMAX[:], HS[:], mybir.AxisListType.X,
                                    Alu.max)
            EQ = pp.tile([P, n_tiles, NBINS], f32)
            hs_b, vmax_b = broadcast_tensor_aps(HS[:], VMAX[:])
            nc.vector.tensor_tensor(EQ[:], hs_b, vmax_b, Alu.is_equal)
            nc.vector.tensor_tensor(EQ[:], EQ[:], IOTA64[:], Alu.mult)
            IDX = pp.tile([P, n_tiles, 1], f32)
            nc.vector.tensor_reduce(IDX[:], EQ[:], mybir.AxisListType.X,
                                    Alu.min)
            nc.vector.tensor_scalar(IDX[:], IDX[:], 64.0, None, Alu.add)

            def neighbor_value(shift, wrap_thr, wrap_add, nm):
                IDXN = pp.tile([P, n_tiles, 1], f32, name=f"idxn_{nm}")
                nc.vector.tensor_scalar(IDXN[:], IDX[:], float(shift), None,
                                        Alu.add)
                WADJ = pp.tile([P, n_tiles, 1], f32, name=f"wadj_{nm}")
                if wrap_add < 0:
                    nc.vector.tensor_scalar(WADJ[:], IDXN[:], wrap_thr,
                                            float(wrap_add), Alu.is_gt,
                                            Alu.mult)
                else:
                    nc.vector.tensor_scalar(WADJ[:], IDXN[:], wrap_thr,
                                            float(wrap_add), Alu.is_lt,
                                            Alu.mult)
                nc.vector.tensor_tensor(IDXN[:], IDXN[:], WADJ[:], Alu.add)
                DIF = pp.tile([P, n_tiles, NBINS], f32, name=f"dif_{nm}")
                iota_b, idxn_b = broadcast_tensor_aps(IOTA[:], IDXN[:])
                nc.vector.tensor_tensor(DIF[:], iota_b, idxn_b, Alu.subtract)
                nc.vector.tensor_scalar(DIF[:], DIF[:], 0.0, None,
                                        Alu.is_equal)
                nc.vector.tensor_tensor(DIF[:], DIF[:], HS[:], Alu.mult)
                V = pp.tile([P, n_tiles, 1], f32, name=f"v_{nm}")
                nc.vector.tensor_reduce(V[:], DIF[:], mybir.AxisListType.X,
                                        Alu.add)
                return V

            VP = neighbor_value(+1, 35.5, -36.0, "p")
            VM = neighbor_value(-1, -0.5, +36.0, "m")

            NUM = pp.tile([P, n_tiles, 1], f32)
            nc.vector.tensor_tensor(NUM[:], VP[:], VM[:], Alu.subtract)
            SUMN = pp.tile([P, n_tiles, 1], f32)
            nc.vector.tensor_tensor(SUMN[:], VP[:], VM[:], Alu.add)
            DEN = pp.tile([P, n_tiles, 1], f32)
            nc.vector.tensor_scalar(DEN[:], VMAX[:], 2.0, None, Alu.mult)
            nc.vector.tensor_tensor(DEN[:], DEN[:], SUMN[:], Alu.subtract)
            RECD = pp.tile([P, n_tiles, 1], f32)
            SCD = pp.tile([P, n_tiles, 1], f32)
            nc.vector.reciprocal_approx_accurate(RECD[:], DEN[:], SCD[:])
            REF = pp.tile([P, n_tiles, 1], f32)
            nc.vector.scalar_tensor_tensor(
                out=REF[:], in0=NUM[:], scalar=0.5, in1=RECD[:],
                op0=Alu.mult, op1=Alu.mult)
            nc.vector.tensor_tensor(REF[:], IDX[:], REF[:], Alu.add)
            nc.vector.tensor_scalar(ANG[:], REF[:, :, 0], -2.0 * PI / NBINS,
                                    PI, Alu.mult, Alu.add)

            out_view = out_t[:].rearrange("(t p) -> p t", p=P)
            nc.sync.dma_start(out_view, ANG[:])
            if _DEBUG_DUMP:
                nc.sync.dma_start(
                    dbg["rl"][:], RL[:].rearrange("p a b -> p (a b)"))
                nc.sync.dma_start(
                    dbg["hs"][:], HS[:].rearrange("p a b -> p (a b)"))

    nc.compile()
    return nc


def _get_built(b_core, smooth_w, wk_is_ones):
    key = (b_core, tuple(float(x) for x in smooth_w), bool(wk_is_ones))
    if key not in _BUILD_CACHE:
        _BUILD_CACHE[key] = _build(b_core, smooth_w, wk_is_ones)
    return _BUILD_CACHE[key]


# --------------------------------------------------------------------------
# host entry point
# --------------------------------------------------------------------------
def kernel(patch, weight_kernel, smooth_w):
    from concourse import bass_utils

    patch = np.ascontiguousarray(np.asarray(patch, dtype=np.float32))
    weight_kernel = np.asarray(weight_kernel, dtype=np.float32)
    smooth_w = np.asarray(smooth_w, dtype=np.float32)

    B = patch.shape[0]
    assert B % (N_CORES * P) == 0, f"B={B} not divisible by {N_CORES * P}"
    b_core = B // N_CORES
    n_tiles = b_core // P

    wk_is_ones = bool(np.all(weight_kernel == 1.0))
    nc = _get_built(b_core, smooth_w, wk_is_ones)

    x = patch.reshape(N_CORES, b_core, HW)

    iota = np.tile(np.arange(NBINS, dtype=np.float32), n_tiles)
    consts_row = np.concatenate([iota, iota - 64.0]).astype(np.float32)
    consts = np.ascontiguousarray(
        np.broadcast_to(consts_row, (P, consts_row.size)))

    in_maps = []
    for i in range(N_CORES):
        m = {"patch": np.ascontiguousarray(x[i]), "consts": consts}
        if not wk_is_ones:
            m["wk"] = np.ascontiguousarray(
                np.broadcast_to(weight_kernel.reshape(-1), (P, HW)))
        in_maps.append(m)

    res = bass_utils.run_bass_kernel_spmd(nc, in_maps,
                                          core_ids=list(range(N_CORES)))
    out = np.concatenate([r["angle"] for r in res.results])
    return out.astype(np.float32)


# revision 33
# speedup vs baseline: 1.0189x; 1.0189x over previous
"""Trainium2 Bass kernel for CustomizablePatchDominantGradientOrientation.

Pipeline per patch (32x32, fp32):
  sobel (replicate pad, [1,2,1]x[-1,0,1] separable; /8 dropped - the final
  angle is invariant to a global scale on (gx, gy, mag))
  mag = sqrt(gx^2+gy^2+eps'), theta = 2*atan(gy/(mag+gx))  (half-angle atan2)
  soft 36-bin histogram of theta weighted by mag, via the relu-knot
  decomposition: with U = t*M (t = angle in bin units), M = mag,
    R_j = sum relu(U - j*M),  L_j = sum relu(j*M - U)
    hist[k] = second difference of L (bins 0..18) / R (bins 19..35),
    wrap bin 0 += R_17.
  The knot passes run as custom DVE ops on fp16-packed U/M at 2 elem/cycle
  (hand-written 2x_1p uop programs; engine falls back to the 1x program if
  the mem-pattern doesn't qualify).
  circular [w0,w1,w2] smoothing, argmax, parabolic refinement -> angle.

Data parallel: B=32768 patches sharded over 8 NeuronCores (4096 each);
per core 32 tiles of [128 patches x 1024 pixels].  Layout is patch-major:
partitions = patches, free axis = pixels.
"""

import math

import numpy as np

NBINS = 36
PI = math.pi
PATCH = 32
HW = PATCH * PATCH
P = 128          # partitions (patches per tile)
N_CORES = 8
GROUP = 6        # tiles per ACT-table-set phase group
GPSIMD_OFFLOAD = True  # host big contiguous sobel TTs on the Pool engine
_DEBUG_DUMP = False    # add dbg_rl / dbg_hs outputs

_BUILD_CACHE = {}
_OPS_REGISTERED = {}


# --------------------------------------------------------------------------
# custom DVE ops
# --------------------------------------------------------------------------
def _register_custom_ops():
    """Register the custom DVE ops at runtime (row assignment + sha pin,
    exactly what a source-level `OPS.append` would do)."""
    if _OPS_REGISTERED:
        return _OPS_REGISTERED
    from operator import add as _op_add

    import concourse.dve_ops as dve_ops
    from concourse.dve_ops import DveOp, _COMPILE_CACHE
    from concourse.dve_spec import (
        Spec, Src0, Src1, C0, C1, Zero, relu, maxx, lower, _has_src1, sq,
    )
    from concourse.dve_uop import (
        DveOpSpec, UopConfig, UopDpConfig, AluOp, AluInp, DelayInp, InpSel,
        OutSel, OutPath, Trigger, ENABLE, DISABLE, _MAX_LANES,
    )

    # ---- dual-knot uop programs (fp32, 1x) -------------------------------
    # One pass accumulates TWO relu-knot sums:
    #   accA (stage-6 a_flop) = sum relu(U - c0*M)        [R] / relu(c0*M - U) [L]
    #   accB (stage-7 a_flop) = sum relu(U - (c0+c1)*M)   [R] / ...           [L]
    # accB is read by the auto-emitted DVE_READ_ACCUMULATOR2_ANT (accum_out);
    # accA by the KNOT_RDA op below (mimics stock DVE_READ_ACCUMULATOR's
    # program, which reads the stage-6 a_flop).
    # Input slots: 1:SRC_0(U) 2:SRC_1(M) 3:CONST_0(c0) 4:CONST_1(step) 5:ZERO
    # -> lanes 0:U 1:M 2:c0 3:step 4:ZERO; lane5 parks d0.
    def _dual_dp(left, seed):
        Pd = DelayInp.PREV_DELAY
        Ad = DelayInp.PREV_ALU_OUT
        I = AluInp

        def blk(op, s0, s1, park=None, a_out=False):
            d = [Pd] * 6 + [Ad] * (_MAX_LANES - 6)
            if park is not None:
                d[park] = Ad
            c = UopDpConfig(op=op, alu_src0=s0, alu_src1=s1,
                            delay=d,
                            delay_enable=[ENABLE] * 6 + [DISABLE] * (_MAX_LANES - 6),
                            alu_out_enable=ENABLE)
            if a_out:
                c.alu_out_a_enable = ENABLE
            return c

        sub0 = ((I.PREV_DELAY_0, I.PREV_ALU_OUT) if not left
                else (I.PREV_ALU_OUT, I.PREV_DELAY_0))
        # d1 = d0 - q (R) / d0 + q (L)
        d1op, d1s = ((AluOp.SUBTRACT, (I.PREV_DELAY_5, I.PREV_ALU_OUT))
                     if not left else
                     (AluOp.ADD, (I.PREV_ALU_OUT, I.PREV_DELAY_5)))
        dps = [
            blk(AluOp.MULTIPLY, I.PREV_DELAY_2, I.PREV_DELAY_1),           # p = c0*M
            blk(AluOp.SUBTRACT, *sub0),                                    # d0
            blk(AluOp.MULTIPLY, I.PREV_DELAY_3, I.PREV_DELAY_1, park=5),   # q = step*M; park d0
            blk(d1op, *d1s),                                               # d1
            blk(AluOp.MAX, I.PREV_DELAY_5, I.PREV_DELAY_4, park=0),        # r0 = relu(d0); park d1
            blk(AluOp.MAX, I.PREV_DELAY_0, I.PREV_DELAY_4, park=1),        # r1 = relu(d1); park r0
            blk(AluOp.ADD, I.CURR_ALU_OUT, I.PREV_DELAY_1, park=2,
                a_out=True),                                               # accA += r0; park r1
            blk(AluOp.ADD, I.CURR_ALU_OUT, I.PREV_DELAY_2, a_out=True),    # accB += r1
        ]
        if seed:
            for st in (6, 7):
                dps[st].op = AluOp.BYPASS
                dps[st].alu_src0 = AluInp.PREV_DELAY_4
                dps[st].alu_src1 = AluInp.PREV_DELAY_4
        return dps

    def _dual_uops(left):
        """5-state program: seed -> steady (accumulate) -> 4-cycle drain pad
        -> emit accA -> emit accB.  The two accumulator values are the
        instruction's own 2-element dst stream, so no separate accumulator
        read instructions are needed and no engine state must survive across
        instructions (the scheduler may interleave anything)."""
        inp = [InpSel.ZERO] * 8
        inp_en = [DISABLE] * 8
        for slot, sel in ((1, InpSel.SRC_0), (2, InpSel.SRC_1),
                          (3, InpSel.CONST_0), (4, InpSel.CONST_1),
                          (5, InpSel.ZERO)):
            inp[slot] = sel
            inp_en[slot] = ENABLE
        seed = UopConfig(
            inp=list(inp), inp_enable=list(inp_en),
            trigger=(Trigger.COUNT, Trigger.NONE, Trigger.NONE),
            next_uop=(1, 0, 0), repeat_count=1, accum_enabled=ENABLE,
            datapath_config=_dual_dp(left, seed=True))
        steady = UopConfig(
            inp=list(inp), inp_enable=list(inp_en),
            trigger=(Trigger.SRC_TENSOR_DONE, Trigger.NONE, Trigger.NONE),
            next_uop=(2, 0, 0), repeat_count=0,
            require_inp0=ENABLE, require_inp1=ENABLE, accum_enabled=ENABLE,
            datapath_config=_dual_dp(left, seed=False))

        def _bypass_dps(read_stage=None):
            dps = []
            for st in range(8):
                c = UopDpConfig(op=AluOp.BYPASS,
                                alu_src0=AluInp.PREV_ALU_OUT,
                                alu_src1=AluInp.PREV_ALU_OUT,
                                delay=[DelayInp.PREV_DELAY] * _MAX_LANES,
                                delay_enable=[DISABLE] * _MAX_LANES,
                                alu_out_enable=ENABLE)
                dps.append(c)
            if read_stage is not None:
                dps[read_stage].alu_src0 = AluInp.NEXT_ALU_OUT_A
                dps[read_stage].alu_src1 = AluInp.NEXT_ALU_OUT_A
            return dps

        pad = UopConfig(
            inp=list(inp), inp_enable=list(inp_en),
            trigger=(Trigger.COUNT, Trigger.NONE, Trigger.NONE),
            next_uop=(3, 0, 0), repeat_count=4, accum_enabled=ENABLE,
            datapath_config=_bypass_dps())
        fin_a = UopConfig(
            inp=list(inp), inp_enable=list(inp_en),
            trigger=(Trigger.COUNT, Trigger.NONE, Trigger.NONE),
            next_uop=(4, 0, 0), repeat_count=1, accum_enabled=ENABLE,
            datapath_config=_bypass_dps(read_stage=5))
        fin_a.enable_output(OutSel.ALU_OUT, OutPath.WR0_LO)
        fin_b = UopConfig(
            inp=list(inp), inp_enable=list(inp_en),
            trigger=(Trigger.COUNT, Trigger.NONE, Trigger.NONE),
            next_uop=(0, 0, 0), repeat_count=1, accum_enabled=ENABLE,
            datapath_config=_bypass_dps(read_stage=6))
        fin_b.enable_output(OutSel.ALU_OUT, OutPath.WR0_LO)
        return [seed, steady, pad, fin_a, fin_b]

    def _rd2_uops():
        """Read BOTH a_flops in one 1-cycle op and write them as two
        consecutive dst elements: dst[0] = accA (stage-6 a_flop, via WR0_LO),
        dst[1] = accB (stage-7 a_flop, via WR1_LO)."""
        Pd = DelayInp.PREV_DELAY
        I = AluInp
        dps = []
        for st in range(8):
            c = UopDpConfig(op=AluOp.BYPASS,
                            alu_src0=I.PREV_ALU_OUT, alu_src1=I.PREV_ALU_OUT,
                            delay=[Pd] * _MAX_LANES,
                            delay_enable=[DISABLE] * _MAX_LANES,
                            alu_out_enable=ENABLE)
            dps.append(c)
        dps[5].alu_src0 = I.NEXT_ALU_OUT_A       # accA (stage-6 a_flop)
        dps[5].alu_src1 = I.NEXT_ALU_OUT_A
        dps[6].alu_src0 = I.NEXT_ALU_OUT_A       # accB (stage-7 a_flop)
        dps[6].alu_src1 = I.NEXT_ALU_OUT_A
        dps[6].delay = [DelayInp.PREV_ALU_OUT] + [Pd] * (_MAX_LANES - 1)
        dps[6].delay_enable = [ENABLE] + [DISABLE] * (_MAX_LANES - 1)
        dps[7].delay_enable = [ENABLE] + [DISABLE] * (_MAX_LANES - 1)
        u = UopConfig(
            inp=[InpSel.ZERO] * 8,
            inp_enable=[DISABLE] * 8,
            trigger=(Trigger.COUNT, Trigger.NONE, Trigger.NONE),
            next_uop=(0, 0, 0), repeat_count=1,
            require_inp0=ENABLE,  # consume the 1-elem in0 (no rd-FIFO leak)
            datapath_config=dps)
        u.enable_input(InpSel.SRC_0, 1)
        u.enable_output(OutSel.DELAY_0, OutPath.WR0_LO)   # accA
        u.enable_output(OutSel.ALU_OUT, OutPath.WR1_LO)   # accB
        return [u]

    def _rda_uops():
        """Read the stage-6 a_flop (accA) and write it as a [P,1] scalar —
        the stock DVE_READ_ACCUMULATOR program (opcode-table slot 117)
        rebuilt as a custom row: one COUNT cycle, stage-5 ALU passes
        NEXT_ALU_OUT_A (= stage-6 a_flop), bypass chain to the writer."""
        Pd = DelayInp.PREV_DELAY
        I = AluInp
        dps = []
        for st in range(8):
            c = UopDpConfig(op=AluOp.BYPASS,
                            alu_src0=I.PREV_ALU_OUT, alu_src1=I.PREV_ALU_OUT,
                            delay=[Pd] * 6 + [DelayInp.PREV_ALU_OUT] * (_MAX_LANES - 6),
                            delay_enable=[DISABLE] * _MAX_LANES,
                            alu_out_enable=ENABLE)
            dps.append(c)
        dps[5].alu_src0 = I.NEXT_ALU_OUT_A
        dps[5].alu_src1 = I.NEXT_ALU_OUT_A
        u = UopConfig(
            inp=[InpSel.ZERO] * 8,
            inp_enable=[DISABLE] * 8,
            trigger=(Trigger.COUNT, Trigger.NONE, Trigger.NONE),
            next_uop=(0, 0, 0), repeat_count=1,
            require_inp0=ENABLE,  # consume the 1-elem in0 (no rd-FIFO leak)
            datapath_config=dps)
        u.enable_input(InpSel.SRC_0, 1)
        u.enable_output(OutSel.ALU_OUT, OutPath.WR0_LO)
        return [u]

    def _ref_dual_r(in0, in1, s0, s1, imm2):
        b = np.maximum(in0 - s0 * in1, 0.0).astype(np.float32)
        b2 = np.maximum(in0 - (s0 + s1) * in1, 0.0).astype(np.float32)
        return b, b2.reshape(b2.shape[0], -1).sum(axis=-1, keepdims=True)

    def _ref_dual_l(in0, in1, s0, s1, imm2):
        b = np.maximum(s0 * in1 - in0, 0.0).astype(np.float32)
        b2 = np.maximum((s0 + s1) * in1 - in0, 0.0).astype(np.float32)
        return b, b2.reshape(b2.shape[0], -1).sum(axis=-1, keepdims=True)

    def _ref_rda(in0, in1, s0, s1, imm2):
        return in0.astype(np.float32)

    def _rsqrt_nr_ref(in0, in1, s0, s1, imm2):
        return ((s0 - in0 * in1 * in1 * s1) * in1).astype(np.float32)

    def _addmax_ref(in0, in1, s0, s1, imm2):
        return np.maximum(in0 + in1, s0).astype(np.float32)

    def _reg(name, spec, uops=None, rd1_en=None):
        if name in dve_ops._SUB_OPCODE_FOR_NAME:
            for op in dve_ops.OPS:
                if op.name == name:
                    return op
        row = dve_ops._CUSTOM_DVE_ROW_BASE + len(dve_ops.OPS)
        assert row < 0x20, "custom-DVE row budget exhausted"
        dve_ops._SUB_OPCODE_FOR_NAME[name] = row
        shas = {}
        for ver in ("v3", "v4"):
            s = DveOpSpec(name=name, opcode=row,
                          uops=(uops if uops is not None
                                else lower(spec, ver=ver)),
                          rd1_en=(rd1_en if rd1_en is not None
                                  else _has_src1(spec)))
            s.validate(ver)
            shas[ver] = s.sha(ver)
            _COMPILE_CACHE[(name, ver)] = s
        op = DveOp(name, spec, subdim=False, uops_sha=shas)
        dve_ops.OPS.append(op)
        dve_ops.CUSTOM_DVE_SPECS[name] = spec
        return op

    _OPS_REGISTERED["knot2_r"] = _reg(
        "KNOT2_R_ANT",
        Spec(body=relu(Src0 - C0 * Src1), accum=_op_add, accum_init=Zero,
             reference=_ref_dual_r),
        uops=_dual_uops(False), rd1_en=True)
    _OPS_REGISTERED["knot2_l"] = _reg(
        "KNOT2_L_ANT",
        Spec(body=relu(C0 * Src1 - Src0), accum=_op_add, accum_init=Zero,
             reference=_ref_dual_l),
        uops=_dual_uops(True), rd1_en=True)
    _OPS_REGISTERED["rda"] = _reg(
        "KNOT_RDA_ANT",
        Spec(body=Src0 * C0, reference=_ref_rda),
        uops=_rda_uops(), rd1_en=False)
    _OPS_REGISTERED["rd2"] = _reg(
        "KNOT_RD2_ANT",
        Spec(body=Src0 * C0, reference=_ref_rda),
        uops=_rd2_uops(), rd1_en=False)
    # z1 = (c0 - g2*z0^2*c1)*z0  (one Newton step toward 1/sqrt(g2))
    _OPS_REGISTERED["rsqrt_nr"] = _reg(
        "RSQRT_NR_ANT",
        Spec(body=(C0 - Src0 * sq(Src1) * C1) * Src1,
             reference=_rsqrt_nr_ref))
    _OPS_REGISTERED["addmax"] = _reg(
        "ADD_MAX_ANT",
        Spec(body=maxx(Src0 + Src1, C0), reference=_addmax_ref))
    return _OPS_REGISTERED


# --------------------------------------------------------------------------
# kernel build
# --------------------------------------------------------------------------
def _build(b_core, smooth_w, wk_is_ones):
    import concourse.bacc as bacc
    import concourse.mybir as mybir
    from concourse.tile import TileContext
    from concourse.bass import broadcast_tensor_aps

    ops = _register_custom_ops()
    KNOT2_R, KNOT2_L, RDA = ops["knot2_r"], ops["knot2_l"], ops["rda"]
    RSQRT_NR, ADDMAX = ops["rsqrt_nr"], ops["addmax"]
    # NEFF-cache salt: the uop tables are side data the compile cache does not
    # key on; bake their hash into an unused immediate so table edits recompile.
    salt = float(sum(int(op.uops_sha["v3"], 16) for op in ops.values())
                 % 1000003) / 1000.0

    f32 = mybir.dt.float32
    f16 = mybir.dt.float16
    Alu = mybir.AluOpType
    Act = mybir.ActivationFunctionType

    n_tiles = b_core // P
    assert b_core % P == 0
    w0, w1, w2 = (float(x) for x in smooth_w)

    nc = bacc.Bacc(None, target_bir_lowering=False, debug=False)
    patch_in = nc.dram_tensor("patch", [b_core, HW], f32, kind="ExternalInput")
    # consts: iota36 repeated n_tiles times, then (iota36 - 64) repeated
    consts_in = nc.dram_tensor("consts", [P, 2 * n_tiles * NBINS], f32,
                               kind="ExternalInput")
    wk_in = None
    if not wk_is_ones:
        wk_in = nc.dram_tensor("wk", [P, HW], f32, kind="ExternalInput")
    out_t = nc.dram_tensor("angle", [b_core], f32, kind="ExternalOutput")
    dbg = {}
    if _DEBUG_DUMP:
        dbg["rl"] = nc.dram_tensor("dbg_rl", [P, n_tiles * 41], f32,
                                   kind="ExternalOutput")
        dbg["hs"] = nc.dram_tensor("dbg_hs", [P, n_tiles * NBINS], f32,
                                   kind="ExternalOutput")

    with TileContext(nc) as tc:
        with tc.tile_pool(name="pool", bufs=2) as pool, \
             tc.tile_pool(name="persist", bufs=1) as pp:
            IOTA = pp.tile([P, n_tiles, NBINS], f32)
            IOTA64 = pp.tile([P, n_tiles, NBINS], f32)
            nc.sync.dma_start(IOTA[:], consts_in[:, 0:n_tiles * NBINS])
            nc.sync.dma_start(IOTA64[:], consts_in[:, n_tiles * NBINS:])
            WK = None
            if wk_in is not None:
                WK = pp.tile([P, HW], f32)
                nc.sync.dma_start(WK[:], wk_in[:])

            # knot sums: L_j (j=-19..1) at 0..20, R_j (j=0..18) at 21..39,
            # slot 40 = discard (unused second knot of the last L pair).
            # L_-19 = L_-18 = 0 (t > -18 always) and R_18 = 0 (t <= 18):
            # those slots stay at the memset value and their passes are skipped.
            RL = pp.tile([P, n_tiles, 41], f32)
            nc.vector.memset(RL[:], 0.0)
            HISTE = pp.tile([P, n_tiles, NBINS + 2], f32)
            ANG = pp.tile([P, n_tiles], f32)

            n_groups = (n_tiles + GROUP - 1) // GROUP
            for g in range(n_groups):
                tiles = range(g * GROUP, min((g + 1) * GROUP, n_tiles))
                slot = {}
                # ---- phase A: sobel, magnitude (sqrt table set) ----
                for t in tiles:
                    s = t % GROUP
                    X = pool.tile([P, HW], f32, tag="x", bufs=3, name=f"x{t}")
                    nc.sync.dma_start(X[:], patch_in[t * P:(t + 1) * P, :])
                    X3 = X.rearrange("p (r c) -> p r c", c=PATCH)

                    SV = pool.tile([P, HW], f32, tag="sv", bufs=3, name=f"sv{t}")
                    # vertical [1,2,1] with replicate rows
                    nc.vector.scalar_tensor_tensor(
                        out=SV[:, 32:992], in0=X[:, 32:992], scalar=2.0,
                        in1=X[:, 0:960], op0=Alu.mult, op1=Alu.add)
                    sv_eng = nc.gpsimd if GPSIMD_OFFLOAD else nc.vector
                    sv_eng.tensor_tensor(
                        SV[:, 32:992], SV[:, 32:992], X[:, 64:1024], Alu.add)
                    nc.vector.scalar_tensor_tensor(
                        out=SV[:, 0:32], in0=X[:, 0:32], scalar=3.0,
                        in1=X[:, 32:64], op0=Alu.mult, op1=Alu.add)
                    nc.vector.scalar_tensor_tensor(
                        out=SV[:, 992:1024], in0=X[:, 992:1024], scalar=3.0,
                        in1=X[:, 960:992], op0=Alu.mult, op1=Alu.add)
                    SV3 = SV.rearrange("p (r c) -> p r c", c=PATCH)

                    GX = pool.tile([P, HW], f32, tag=f"gx{s}", bufs=1,
                                   name=f"gx{t}")
                    GX3 = GX.rearrange("p (r c) -> p r c", c=PATCH)
                    # horizontal central difference with replicate cols
                    nc.vector.tensor_tensor(
                        GX3[:, :, 1:31], SV3[:, :, 2:32], SV3[:, :, 0:30],
                        Alu.subtract)
                    nc.vector.tensor_tensor(
                        GX3[:, :, 0:1], SV3[:, :, 1:2], SV3[:, :, 0:1],
                        Alu.subtract)
                    nc.vector.tensor_tensor(
                        GX3[:, :, 31:32], SV3[:, :, 31:32], SV3[:, :, 30:31],
                        Alu.subtract)

                    SH = pool.tile([P, HW], f32, tag="sh", bufs=3, name=f"sh{t}")
                    SH3 = SH.rearrange("p (r c) -> p r c", c=PATCH)
                    # horizontal [1,2,1] with replicate cols
                    nc.vector.scalar_tensor_tensor(
                        out=SH3[:, :, 1:31], in0=X3[:, :, 1:31], scalar=2.0,
                        in1=X3[:, :, 0:30], op0=Alu.mult, op1=Alu.add)
                    nc.vector.tensor_tensor(
                        SH3[:, :, 1:31], SH3[:, :, 1:31], X3[:, :, 2:32],
                        Alu.add)
                    nc.vector.scalar_tensor_tensor(
                        out=SH3[:, :, 0:1], in0=X3[:, :, 0:1], scalar=3.0,
                        in1=X3[:, :, 1:2], op0=Alu.mult, op1=Alu.add)
                    nc.vector.scalar_tensor_tensor(
                        out=SH3[:, :, 31:32], in0=X3[:, :, 31:32], scalar=3.0,
                        in1=X3[:, :, 30:31], op0=Alu.mult, op1=Alu.add)

                    GY = pool.tile([P, HW], f32, tag=f"gy{s}", bufs=1,
                                   name=f"gy{t}")
                    # vertical central difference with replicate rows
                    gy_eng = nc.gpsimd if GPSIMD_OFFLOAD else nc.vector
                    gy_eng.tensor_tensor(
                        GY[:, 32:992], SH[:, 64:1024], SH[:, 0:960],
                        Alu.subtract)
                    nc.vector.tensor_tensor(
                        GY[:, 0:32], SH[:, 32:64], SH[:, 0:32], Alu.subtract)
                    nc.vector.tensor_tensor(
                        GY[:, 992:1024], SH[:, 992:1024], SH[:, 960:992],
                        Alu.subtract)

                    if WK is not None:
                        nc.vector.tensor_tensor(GX[:], GX[:], WK[:], Alu.mult)
                        nc.vector.tensor_tensor(GY[:], GY[:], WK[:], Alu.mult)

                    # g2 = gx^2 + gy^2 + eps  (eps scaled by 8^2 vs reference)
                    # sv/sh slots are dead here; reuse their tags for squares.
                    # Exact fp32 multiplies on GPSIMD (ACT Square is ~1e-5
                    # off, which poisons the magnitude beyond repair).
                    X2 = pool.tile([P, HW], f32, tag="sv", bufs=3, name=f"x2{t}")
                    Y2 = pool.tile([P, HW], f32, tag="sh", bufs=3, name=f"y2{t}")
                    nc.gpsimd.tensor_tensor(X2[:], GX[:], GX[:], Alu.mult)
                    nc.gpsimd.tensor_tensor(Y2[:], GY[:], GY[:], Alu.mult)
                    G2 = pool.tile([P, HW], f32, tag="g2", name=f"g2{t}")
                    nc.vector.scalar_tensor_tensor(
                        out=G2[:], in0=X2[:], scalar=6.4e-17, in1=Y2[:],
                        op0=Alu.add, op1=Alu.add)
                    M = pool.tile([P, HW], f32, tag=f"m{s}", bufs=1,
                                   name=f"m{t}")
                    nc.scalar.activation(M[:], G2[:], Act.Sqrt)
                    # one Newton step: m = g2 * nr(1/m0)
                    RC = pool.tile([P, HW], f32, tag="rc", name=f"rc{t}")
                    SC = pool.tile([P, HW], f32, tag="sc", name=f"sc{t}")
                    nc.vector.reciprocal_approx_fast(RC[:], M[:])
                    nc.vector._custom_dve(RSQRT_NR, out=SC[:], in0=G2[:],
                                          in1=RC[:], s0=1.5, s1=0.5)
                    nc.vector.tensor_tensor(M[:], G2[:], SC[:], Alu.mult)
                    slot[t] = (GX, GY, M)

                # ---- phase B: orientation + knot histogram (arctan set) --
                for t in tiles:
                    GX, GY, M = slot[t]
                    # d = max(m + gx, 1e-30): the clamp both avoids the
                    # recip(0)=NaN edge and pins rounding-negative d to the
                    # correct wrap side (t -> 36/0 by sign of gy).
                    D = pool.tile([P, HW], f32, tag="g2", name=f"d{t}")
                    nc.vector._custom_dve(ADDMAX, out=D[:], in0=M[:],
                                          in1=GX[:], s0=1e-30)
                    RC = pool.tile([P, HW], f32, tag="rc", name=f"rcb{t}")
                    SC = pool.tile([P, HW], f32, tag="sc", name=f"scb{t}")
                    nc.vector.reciprocal_approx_accurate(RC[:], D[:], SC[:])
                    nc.vector.tensor_tensor(SC[:], GY[:], RC[:], Alu.mult)
                    A = pool.tile([P, HW], f32, tag="a", name=f"a{t}")
                    nc.scalar.activation(A[:], SC[:], Act.Arctan)

                    # U = A*M (atan units premultiplied by M); knots j*pi/36.
                    # On DVE: the gpsimd round-trip stalled the dual passes.
                    U = pool.tile([P, HW], f32, tag="u", name=f"u{t}")
                    nc.vector.tensor_tensor(U[:], A[:], M[:], Alu.mult)

                    hb = PI / 36.0  # knot spacing in atan units

                    def dual(op, j0, slotA):
                        # writes (accA, accB) = (knot j0, knot j0+1) as the
                        # instruction's own 2-element dst stream
                        nc.vector._custom_dve(
                            op, out=RL[:, t, slotA:slotA + 2],
                            in0=U[:], in1=M[:],
                            s0=float(j0) * hb, s1=hb, imm2=salt)

                    # L pairs: knots (j, j+1) -> slots (j+19, j+20); the last
                    # pair's L_2 lands on slot 21, overwritten by R_0 below.
                    for j in range(-17, 2, 2):
                        dual(KNOT2_L, j, j + 19)
                    for j in range(0, 18, 2):     # R pairs -> slots 21+j, 22+j
                        dual(KNOT2_R, j, 21 + j)

            # ---- tail: D2, smoothing, argmax, refinement (batched) ----
            # bins 0..18 from L: hist[k] = L[k] - 2 L[k+1] + L[k+2]
            # bins 19..35 from R: hist[k] = R[k-19] - 2 R[k-18] + R[k-17]
            # wrap: hist[0] += R_17
            HC = HISTE[:, :, 1:37]  # core 36 bins
            T1 = pp.tile([P, n_tiles, 19], f32)
            nc.vector.scalar_tensor_tensor(
                out=T1[:], in0=RL[:, :, 1:20], scalar=-2.0,
                in1=RL[:, :, 0:19], op0=Alu.mult, op1=Alu.add)
            nc.vector.tensor_tensor(HC[:, :, 0:19], T1[:], RL[:, :, 2:21],
                                    Alu.add)
            T2 = pp.tile([P, n_tiles, 17], f32)
            nc.vector.scalar_tensor_tensor(
                out=T2[:], in0=RL[:, :, 22:39], scalar=-2.0,
                in1=RL[:, :, 21:38], op0=Alu.mult, op1=Alu.add)
            nc.vector.tensor_tensor(HC[:, :, 19:36], T2[:], RL[:, :, 23:40],
                                    Alu.add)
            nc.vector.tensor_tensor(HC[:, :, 0:1], HC[:, :, 0:1],
                                    RL[:, :, 38:39], Alu.add)

            # wrap columns for circular smoothing
            nc.vector.tensor_copy(HISTE[:, :, 0:1], HISTE[:, :, 36:37])
            nc.vector.tensor_copy(HISTE[:, :, 37:38], HISTE[:, :, 1:2])

            SM = pp.tile([P, n_tiles, NBINS], f32)
            nc.vector.tensor_scalar(SM[:], HISTE[:, :, 2:38], w2, None,
                                    Alu.mult)
            nc.vector.scalar_tensor_tensor(
                out=SM[:], in0=HISTE[:, :, 0:36], scalar=w0, in1=SM[:],
                op0=Alu.mult, op1=Alu.add)
            HS = pp.tile([P, n_tiles, NBINS], f32)
            nc.vector.scalar_tensor_tensor(
                out=HS[:], in0=HISTE[:, :, 1:37], scalar=w1, in1=SM[:],
                op0=Alu.mult, op1=Alu.add)

            VMAX = pp.tile([P, n_tiles, 1], f32)
            nc.vector.tensor_reduce(VMAX[:], HS[:], mybir.AxisListType.X,
                                    Alu.max)
            EQ = pp.tile([P, n_tiles, NBINS], f32)
            hs_b, vmax_b = broadcast_tensor_aps(HS[:], VMAX[:])
            nc.vector.tensor_tensor(EQ[:], hs_b, vmax_b, Alu.is_equal)
            nc.vector.tensor_tensor(EQ[:], EQ[:], IOTA64[:], Alu.mult)
            IDX = pp.tile([P, n_tiles, 1], f32)
            nc.vector.tensor_reduce(IDX[:], EQ[:], mybir.AxisListType.X,
                                    Alu.min)
            nc.vector.tensor_scalar(IDX[:], IDX[:], 64.0, None, Alu.add)

            def neighbor_value(shift, wrap_thr, wrap_add, nm):
                IDXN = pp.tile([P, n_tiles, 1], f32, name=f"idxn_{nm}")
                nc.vector.tensor_scalar(IDXN[:], IDX[:], float(shift), None,
                                        Alu.add)
                WADJ = pp.tile([P, n_tiles, 1], f32, name=f"wadj_{nm}")
                if wrap_add < 0:
                    nc.vector.tensor_scalar(WADJ[:], IDXN[:], wrap_thr,
                                            float(wrap_add), Alu.is_gt,
                                            Alu.mult)
                else:
                    nc.vector.tensor_scalar(WADJ[:], IDXN[:], wrap_thr,
                                            float(wrap_add), Alu.is_lt,
                                            Alu.mult)
                nc.vector.tensor_tensor(IDXN[:], IDXN[:], WADJ[:], Alu.add)
                DIF = pp.tile([P, n_tiles, NBINS], f32, name=f"dif_{nm}")
                iota_b, idxn_b = broadcast_tensor_aps(IOTA[:], IDXN[:])
                nc.vector.tensor_tensor(DIF[:], iota_b, idxn_b, Alu.subtract)
                nc.vector.tensor_scalar(DIF[:], DIF[:], 0.0, None,
                                        Alu.is_equal)
                nc.vector.tensor_tensor(DIF[:], DIF[:], HS[:], Alu.mult)
                V = pp.tile([P, n_tiles, 1], f32, name=f"v_{nm}")
                nc.vector.tensor_reduce(V[:], DIF[:], mybir.AxisListType.X,
                                        Alu.add)
                return V

            VP = neighbor_value(+1, 35.5, -36.0, "p")
            VM = neighbor_value(-1, -0.5, +36.0, "m")

            NUM = pp.tile([P, n_tiles, 1], f32)
            nc.vector.tensor_tensor(NUM[:], VP[:], VM[:], Alu.subtract)
            SUMN = pp.tile([P, n_tiles, 1], f32)
            nc.vector.tensor_tensor(SUMN[:], VP[:], VM[:], Alu.add)
            DEN = pp.tile([P, n_tiles, 1], f32)
            nc.vector.tensor_scalar(DEN[:], VMAX[:], 2.0, None, Alu.mult)
            nc.vector.tensor_tensor(DEN[:], DEN[:], SUMN[:], Alu.subtract)
            RECD = pp.tile([P, n_tiles, 1], f32)
            SCD = pp.tile([P, n_tiles, 1], f32)
            nc.vector.reciprocal_approx_accurate(RECD[:], DEN[:], SCD[:])
            REF = pp.tile([P, n_tiles, 1], f32)
            nc.vector.scalar_tensor_tensor(
                out=REF[:], in0=NUM[:], scalar=0.5, in1=RECD[:],
                op0=Alu.mult, op1=Alu.mult)
            nc.vector.tensor_tensor(REF[:], IDX[:], REF[:], Alu.add)
            nc.vector.tensor_scalar(ANG[:], REF[:, :, 0], -2.0 * PI / NBINS,
                                    PI, Alu.mult, Alu.add)

            out_view = out_t[:].rearrange("(t p) -> p t", p=P)
            nc.sync.dma_start(out_view, ANG[:])
            if _DEBUG_DUMP:
                nc.sync.dma_start(
                    dbg["rl"][:], RL[:].rearrange("p a b -> p (a b)"))
                nc.sync.dma_start(
                    dbg["hs"][:], HS[:].rearrange("p a b -> p (a b)"))

    nc.compile()
    return nc


def _get_built(b_core, smooth_w, wk_is_ones):
    key = (b_core, tuple(float(x) for x in smooth_w), bool(wk_is_ones))
    if key not in _BUILD_CACHE:
        _BUILD_CACHE[key] = _build(b_core, smooth_w, wk_is_ones)
    return _BUILD_CACHE[key]


# --------------------------------------------------------------------------
# host entry point
# --------------------------------------------------------------------------
def kernel(patch, weight_kernel, smooth_w):
    from concourse import bass_utils

    patch = np.ascontiguousarray(np.asarray(patch, dtype=np.float32))
    weight_kernel = np.asarray(weight_kernel, dtype=np.float32)
    smooth_w = np.asarray(smooth_w, dtype=np.float32)

    B = patch.shape[0]
    assert B % (N_CORES * P) == 0, f"B={B} not divisible by {N_CORES * P}"
    b_core = B // N_CORES
    n_tiles = b_core // P

    wk_is_ones = bool(np.all(weight_kernel == 1.0))
    nc = _get_built(b_core, smooth_w, wk_is_ones)

    x = patch.reshape(N_CORES, b_core, HW)

    iota = np.tile(np.arange(NBINS, dtype=np.float32), n_tiles)
    consts_row = np.concatenate([iota, iota - 64.0]).astype(np.float32)
    consts = np.ascontiguousarray(
        np.broadcast_to(consts_row, (P, consts_row.size)))

    in_maps = []
    for i in range(N_CORES):
        m = {"patch": np.ascontiguousarray(x[i]), "consts": consts}
        if not wk_is_ones:
            m["wk"] = np.ascontiguousarray(
                np.broadcast_to(weight_kernel.reshape(-1), (P, HW)))
        in_maps.append(m)

    res = bass_utils.run_bass_kernel_spmd(nc, in_maps,
                                          core_ids=list(range(N_CORES)))
    out = np.concatenate([r["angle"] for r in res.results])
    return out.astype(np.float32)


# revision 40
# speedup vs baseline: 1.0459x; 1.0265x over previous
"""Trainium2 Bass kernel for CustomizablePatchDominantGradientOrientation.

Pipeline per patch (32x32, fp32):
  sobel (replicate pad, [1,2,1]x[-1,0,1] separable; /8 dropped - the final
  angle is invariant to a global scale on (gx, gy, mag))
  mag = sqrt(gx^2+gy^2+eps'), theta = 2*atan(gy/(mag+gx))  (half-angle atan2)
  soft 36-bin histogram of theta weighted by mag, via the relu-knot
  decomposition: with U = t*M (t = angle in bin units), M = mag,
    R_j = sum relu(U - j*M),  L_j = sum relu(j*M - U)
    hist[k] = second difference of L (bins 0..18) / R (bins 19..35),
    wrap bin 0 += R_17.
  The knot passes run as custom DVE ops on fp16-packed U/M at 2 elem/cycle
  (hand-written 2x_1p uop programs; engine falls back to the 1x program if
  the mem-pattern doesn't qualify).
  circular [w0,w1,w2] smoothing, argmax, parabolic refinement -> angle.

Data parallel: B=32768 patches sharded over 8 NeuronCores (4096 each);
per core 32 tiles of [128 patches x 1024 pixels].  Layout is patch-major:
partitions = patches, free axis = pixels.
"""

import math

import numpy as np

NBINS = 36
PI = math.pi
PATCH = 32
HW = PATCH * PATCH
P = 128          # partitions (patches per tile)
N_CORES = 8
GROUP = 6        # tiles per ACT-table-set phase group
GPSIMD_OFFLOAD = True  # host big contiguous sobel TTs on the Pool engine
_DEBUG_DUMP = False    # add dbg_rl / dbg_hs outputs

_BUILD_CACHE = {}
_OPS_REGISTERED = {}


# --------------------------------------------------------------------------
# custom DVE ops
# --------------------------------------------------------------------------
def _register_custom_ops():
    """Register the custom DVE ops at runtime (row assignment + sha pin,
    exactly what a source-level `OPS.append` would do)."""
    if _OPS_REGISTERED:
        return _OPS_REGISTERED
    from operator import add as _op_add

    import concourse.dve_ops as dve_ops
    from concourse.dve_ops import DveOp, _COMPILE_CACHE
    from concourse.dve_spec import (
        Spec, Src0, Src1, C0, C1, Zero, relu, maxx, lower, _has_src1, sq,
    )
    from concourse.dve_uop import (
        DveOpSpec, UopConfig, UopDpConfig, AluOp, AluInp, DelayInp, InpSel,
        OutSel, OutPath, Trigger, ENABLE, DISABLE, _MAX_LANES,
    )

    # ---- dual-knot uop programs (fp32, 1x) -------------------------------
    # One pass accumulates TWO relu-knot sums:
    #   accA (stage-6 a_flop) = sum relu(U - c0*M)        [R] / relu(c0*M - U) [L]
    #   accB (stage-7 a_flop) = sum relu(U - (c0+c1)*M)   [R] / ...           [L]
    # accB is read by the auto-emitted DVE_READ_ACCUMULATOR2_ANT (accum_out);
    # accA by the KNOT_RDA op below (mimics stock DVE_READ_ACCUMULATOR's
    # program, which reads the stage-6 a_flop).
    # Input slots: 1:SRC_0(U) 2:SRC_1(M) 3:CONST_0(c0) 4:CONST_1(step) 5:ZERO
    # -> lanes 0:U 1:M 2:c0 3:step 4:ZERO; lane5 parks d0.
    def _dual_dp(left, seed):
        Pd = DelayInp.PREV_DELAY
        Ad = DelayInp.PREV_ALU_OUT
        I = AluInp

        def blk(op, s0, s1, park=None, a_out=False):
            d = [Pd] * 6 + [Ad] * (_MAX_LANES - 6)
            if park is not None:
                d[park] = Ad
            c = UopDpConfig(op=op, alu_src0=s0, alu_src1=s1,
                            delay=d,
                            delay_enable=[ENABLE] * 6 + [DISABLE] * (_MAX_LANES - 6),
                            alu_out_enable=ENABLE)
            if a_out:
                c.alu_out_a_enable = ENABLE
            return c

        sub0 = ((I.PREV_DELAY_0, I.PREV_ALU_OUT) if not left
                else (I.PREV_ALU_OUT, I.PREV_DELAY_0))
        # d1 = d0 - q (R) / d0 + q (L)
        d1op, d1s = ((AluOp.SUBTRACT, (I.PREV_DELAY_5, I.PREV_ALU_OUT))
                     if not left else
                     (AluOp.ADD, (I.PREV_ALU_OUT, I.PREV_DELAY_5)))
        dps = [
            blk(AluOp.MULTIPLY, I.PREV_DELAY_2, I.PREV_DELAY_1),           # p = c0*M
            blk(AluOp.SUBTRACT, *sub0),                                    # d0
            blk(AluOp.MULTIPLY, I.PREV_DELAY_3, I.PREV_DELAY_1, park=5),   # q = step*M; park d0
            blk(d1op, *d1s),                                               # d1
            blk(AluOp.MAX, I.PREV_DELAY_5, I.PREV_DELAY_4, park=0),        # r0 = relu(d0); park d1
            blk(AluOp.MAX, I.PREV_DELAY_0, I.PREV_DELAY_4, park=1),        # r1 = relu(d1); park r0
            blk(AluOp.ADD, I.CURR_ALU_OUT, I.PREV_DELAY_1, park=2,
                a_out=True),                                               # accA += r0; park r1
            blk(AluOp.ADD, I.CURR_ALU_OUT, I.PREV_DELAY_2, a_out=True),    # accB += r1
        ]
        if seed:
            for st in (6, 7):
                dps[st].op = AluOp.BYPASS
                dps[st].alu_src0 = AluInp.PREV_DELAY_4
                dps[st].alu_src1 = AluInp.PREV_DELAY_4
        return dps

    def _dual_uops(left):
        """5-state program: seed -> steady (accumulate) -> 4-cycle drain pad
        -> emit accA -> emit accB.  The two accumulator values are the
        instruction's own 2-element dst stream, so no separate accumulator
        read instructions are needed and no engine state must survive across
        instructions (the scheduler may interleave anything)."""
        inp = [InpSel.ZERO] * 8
        inp_en = [DISABLE] * 8
        for slot, sel in ((1, InpSel.SRC_0), (2, InpSel.SRC_1),
                          (3, InpSel.CONST_0), (4, InpSel.CONST_1),
                          (5, InpSel.ZERO)):
            inp[slot] = sel
            inp_en[slot] = ENABLE
        seed = UopConfig(
            inp=list(inp), inp_enable=list(inp_en),
            trigger=(Trigger.COUNT, Trigger.NONE, Trigger.NONE),
            next_uop=(1, 0, 0), repeat_count=1, accum_enabled=ENABLE,
            datapath_config=_dual_dp(left, seed=True))
        steady = UopConfig(
            inp=list(inp), inp_enable=list(inp_en),
            trigger=(Trigger.SRC_TENSOR_DONE, Trigger.NONE, Trigger.NONE),
            next_uop=(2, 0, 0), repeat_count=0,
            require_inp0=ENABLE, require_inp1=ENABLE, accum_enabled=ENABLE,
            datapath_config=_dual_dp(left, seed=False))

        def _bypass_dps(read_stage=None):
            dps = []
            for st in range(8):
                c = UopDpConfig(op=AluOp.BYPASS,
                                alu_src0=AluInp.PREV_ALU_OUT,
                                alu_src1=AluInp.PREV_ALU_OUT,
                                delay=[DelayInp.PREV_DELAY] * _MAX_LANES,
                                delay_enable=[DISABLE] * _MAX_LANES,
                                alu_out_enable=ENABLE)
                dps.append(c)
            if read_stage is not None:
                dps[read_stage].alu_src0 = AluInp.NEXT_ALU_OUT_A
                dps[read_stage].alu_src1 = AluInp.NEXT_ALU_OUT_A
            return dps

        pad = UopConfig(
            inp=list(inp), inp_enable=list(inp_en),
            trigger=(Trigger.COUNT, Trigger.NONE, Trigger.NONE),
            next_uop=(3, 0, 0), repeat_count=4, accum_enabled=ENABLE,
            datapath_config=_bypass_dps())
        fin_a = UopConfig(
            inp=list(inp), inp_enable=list(inp_en),
            trigger=(Trigger.COUNT, Trigger.NONE, Trigger.NONE),
            next_uop=(4, 0, 0), repeat_count=1, accum_enabled=ENABLE,
            datapath_config=_bypass_dps(read_stage=5))
        fin_a.enable_output(OutSel.ALU_OUT, OutPath.WR0_LO)
        fin_b = UopConfig(
            inp=list(inp), inp_enable=list(inp_en),
            trigger=(Trigger.COUNT, Trigger.NONE, Trigger.NONE),
            next_uop=(0, 0, 0), repeat_count=1, accum_enabled=ENABLE,
            datapath_config=_bypass_dps(read_stage=6))
        fin_b.enable_output(OutSel.ALU_OUT, OutPath.WR0_LO)
        return [seed, steady, pad, fin_a, fin_b]

    def _rd2_uops():
        """Read BOTH a_flops in one 1-cycle op and write them as two
        consecutive dst elements: dst[0] = accA (stage-6 a_flop, via WR0_LO),
        dst[1] = accB (stage-7 a_flop, via WR1_LO)."""
        Pd = DelayInp.PREV_DELAY
        I = AluInp
        dps = []
        for st in range(8):
            c = UopDpConfig(op=AluOp.BYPASS,
                            alu_src0=I.PREV_ALU_OUT, alu_src1=I.PREV_ALU_OUT,
                            delay=[Pd] * _MAX_LANES,
                            delay_enable=[DISABLE] * _MAX_LANES,
                            alu_out_enable=ENABLE)
            dps.append(c)
        dps[5].alu_src0 = I.NEXT_ALU_OUT_A       # accA (stage-6 a_flop)
        dps[5].alu_src1 = I.NEXT_ALU_OUT_A
        dps[6].alu_src0 = I.NEXT_ALU_OUT_A       # accB (stage-7 a_flop)
        dps[6].alu_src1 = I.NEXT_ALU_OUT_A
        dps[6].delay = [DelayInp.PREV_ALU_OUT] + [Pd] * (_MAX_LANES - 1)
        dps[6].delay_enable = [ENABLE] + [DISABLE] * (_MAX_LANES - 1)
        dps[7].delay_enable = [ENABLE] + [DISABLE] * (_MAX_LANES - 1)
        u = UopConfig(
            inp=[InpSel.ZERO] * 8,
            inp_enable=[DISABLE] * 8,
            trigger=(Trigger.COUNT, Trigger.NONE, Trigger.NONE),
            next_uop=(0, 0, 0), repeat_count=1,
            require_inp0=ENABLE,  # consume the 1-elem in0 (no rd-FIFO leak)
            datapath_config=dps)
        u.enable_input(InpSel.SRC_0, 1)
        u.enable_output(OutSel.DELAY_0, OutPath.WR0_LO)   # accA
        u.enable_output(OutSel.ALU_OUT, OutPath.WR1_LO)   # accB
        return [u]

    def _rda_uops():
        """Read the stage-6 a_flop (accA) and write it as a [P,1] scalar —
        the stock DVE_READ_ACCUMULATOR program (opcode-table slot 117)
        rebuilt as a custom row: one COUNT cycle, stage-5 ALU passes
        NEXT_ALU_OUT_A (= stage-6 a_flop), bypass chain to the writer."""
        Pd = DelayInp.PREV_DELAY
        I = AluInp
        dps = []
        for st in range(8):
            c = UopDpConfig(op=AluOp.BYPASS,
                            alu_src0=I.PREV_ALU_OUT, alu_src1=I.PREV_ALU_OUT,
                            delay=[Pd] * 6 + [DelayInp.PREV_ALU_OUT] * (_MAX_LANES - 6),
                            delay_enable=[DISABLE] * _MAX_LANES,
                            alu_out_enable=ENABLE)
            dps.append(c)
        dps[5].alu_src0 = I.NEXT_ALU_OUT_A
        dps[5].alu_src1 = I.NEXT_ALU_OUT_A
        u = UopConfig(
            inp=[InpSel.ZERO] * 8,
            inp_enable=[DISABLE] * 8,
            trigger=(Trigger.COUNT, Trigger.NONE, Trigger.NONE),
            next_uop=(0, 0, 0), repeat_count=1,
            require_inp0=ENABLE,  # consume the 1-elem in0 (no rd-FIFO leak)
            datapath_config=dps)
        u.enable_input(InpSel.SRC_0, 1)
        u.enable_output(OutSel.ALU_OUT, OutPath.WR0_LO)
        return [u]

    def _ref_dual_r(in0, in1, s0, s1, imm2):
        b = np.maximum(in0 - s0 * in1, 0.0).astype(np.float32)
        b2 = np.maximum(in0 - (s0 + s1) * in1, 0.0).astype(np.float32)
        return b, b2.reshape(b2.shape[0], -1).sum(axis=-1, keepdims=True)

    def _ref_dual_l(in0, in1, s0, s1, imm2):
        b = np.maximum(s0 * in1 - in0, 0.0).astype(np.float32)
        b2 = np.maximum((s0 + s1) * in1 - in0, 0.0).astype(np.float32)
        return b, b2.reshape(b2.shape[0], -1).sum(axis=-1, keepdims=True)

    def _ref_rda(in0, in1, s0, s1, imm2):
        return in0.astype(np.float32)

    def _rsqrt_nr_ref(in0, in1, s0, s1, imm2):
        return ((s0 - in0 * in1 * in1 * s1) * in1).astype(np.float32)

    def _rsqrt_nr_mul_ref(in0, in1, s0, s1, imm2):
        # m = g2 * ((s0 - g2*rc^2*s1) * rc)  -- NR toward 1/sqrt(g2), then *g2
        return (in0 * ((s0 - in0 * in1 * in1 * s1) * in1)).astype(np.float32)

    def _addmax_ref(in0, in1, s0, s1, imm2):
        return np.maximum(in0 + in1, s0).astype(np.float32)

    def _reg(name, spec, uops=None, rd1_en=None):
        if name in dve_ops._SUB_OPCODE_FOR_NAME:
            for op in dve_ops.OPS:
                if op.name == name:
                    return op
        row = dve_ops._CUSTOM_DVE_ROW_BASE + len(dve_ops.OPS)
        assert row < 0x20, "custom-DVE row budget exhausted"
        dve_ops._SUB_OPCODE_FOR_NAME[name] = row
        shas = {}
        for ver in ("v3", "v4"):
            s = DveOpSpec(name=name, opcode=row,
                          uops=(uops if uops is not None
                                else lower(spec, ver=ver)),
                          rd1_en=(rd1_en if rd1_en is not None
                                  else _has_src1(spec)))
            s.validate(ver)
            shas[ver] = s.sha(ver)
            _COMPILE_CACHE[(name, ver)] = s
        op = DveOp(name, spec, subdim=False, uops_sha=shas)
        dve_ops.OPS.append(op)
        dve_ops.CUSTOM_DVE_SPECS[name] = spec
        return op

    _OPS_REGISTERED["knot2_r"] = _reg(
        "KNOT2_R_ANT",
        Spec(body=relu(Src0 - C0 * Src1), accum=_op_add, accum_init=Zero,
             reference=_ref_dual_r),
        uops=_dual_uops(False), rd1_en=True)
    _OPS_REGISTERED["knot2_l"] = _reg(
        "KNOT2_L_ANT",
        Spec(body=relu(C0 * Src1 - Src0), accum=_op_add, accum_init=Zero,
             reference=_ref_dual_l),
        uops=_dual_uops(True), rd1_en=True)
    _OPS_REGISTERED["rda"] = _reg(
        "KNOT_RDA_ANT",
        Spec(body=Src0 * C0, reference=_ref_rda),
        uops=_rda_uops(), rd1_en=False)
    _OPS_REGISTERED["rd2"] = _reg(
        "KNOT_RD2_ANT",
        Spec(body=Src0 * C0, reference=_ref_rda),
        uops=_rd2_uops(), rd1_en=False)
    # z1 = (c0 - g2*z0^2*c1)*z0  (one Newton step toward 1/sqrt(g2))
    _OPS_REGISTERED["rsqrt_nr"] = _reg(
        "RSQRT_NR_ANT",
        Spec(body=(C0 - Src0 * sq(Src1) * C1) * Src1,
             reference=_rsqrt_nr_ref))
    # m = g2 * nr(...)  (the refined-sqrt multiply fused in)
    _OPS_REGISTERED["rsqrt_nr_mul"] = _reg(
        "RSQRT_NR_MUL_ANT",
        Spec(body=(C0 - Src0 * sq(Src1) * C1) * Src1 * Src0,
             reference=_rsqrt_nr_mul_ref))
    _OPS_REGISTERED["addmax"] = _reg(
        "ADD_MAX_ANT",
        Spec(body=maxx(Src0 + Src1, C0), reference=_addmax_ref))
    return _OPS_REGISTERED


# --------------------------------------------------------------------------
# kernel build
# --------------------------------------------------------------------------
def _build(b_core, smooth_w, wk_is_ones):
    import concourse.bacc as bacc
    import concourse.mybir as mybir
    from concourse.tile import TileContext
    from concourse.bass import broadcast_tensor_aps

    ops = _register_custom_ops()
    KNOT2_R, KNOT2_L = ops["knot2_r"], ops["knot2_l"]
    RSQRT_NR_MUL, ADDMAX = ops["rsqrt_nr_mul"], ops["addmax"]
    # NEFF-cache salt: the uop tables are side data the compile cache does not
    # key on; bake their hash into an unused immediate so table edits recompile.
    salt = float(sum(int(op.uops_sha["v3"], 16) for op in ops.values())
                 % 1000003) / 1000.0

    f32 = mybir.dt.float32
    f16 = mybir.dt.float16
    Alu = mybir.AluOpType
    Act = mybir.ActivationFunctionType

    n_tiles = b_core // P
    assert b_core % P == 0
    w0, w1, w2 = (float(x) for x in smooth_w)

    nc = bacc.Bacc(None, target_bir_lowering=False, debug=False)
    patch_in = nc.dram_tensor("patch", [b_core, HW], f32, kind="ExternalInput")
    # consts: iota36 repeated n_tiles times, then (iota36 - 64) repeated
    consts_in = nc.dram_tensor("consts", [P, 2 * n_tiles * NBINS], f32,
                               kind="ExternalInput")
    wk_in = None
    if not wk_is_ones:
        wk_in = nc.dram_tensor("wk", [P, HW], f32, kind="ExternalInput")
    out_t = nc.dram_tensor("angle", [b_core], f32, kind="ExternalOutput")
    dbg = {}
    if _DEBUG_DUMP:
        dbg["rl"] = nc.dram_tensor("dbg_rl", [P, n_tiles * 41], f32,
                                   kind="ExternalOutput")
        dbg["hs"] = nc.dram_tensor("dbg_hs", [P, n_tiles * NBINS], f32,
                                   kind="ExternalOutput")

    with TileContext(nc) as tc:
        with tc.tile_pool(name="pool", bufs=2) as pool, \
             tc.tile_pool(name="persist", bufs=1) as pp:
            IOTA = pp.tile([P, n_tiles, NBINS], f32)
            IOTA64 = pp.tile([P, n_tiles, NBINS], f32)
            nc.sync.dma_start(IOTA[:], consts_in[:, 0:n_tiles * NBINS])
            nc.sync.dma_start(IOTA64[:], consts_in[:, n_tiles * NBINS:])
            WK = None
            if wk_in is not None:
                WK = pp.tile([P, HW], f32)
                nc.sync.dma_start(WK[:], wk_in[:])

            # knot sums: L_j (j=-19..1) at 0..20, R_j (j=0..18) at 21..39,
            # slot 40 = discard (unused second knot of the last L pair).
            # L_-19 = L_-18 = 0 (t > -18 always) and R_18 = 0 (t <= 18):
            # those slots stay at the memset value and their passes are skipped.
            RL = pp.tile([P, n_tiles, 41], f32)
            nc.vector.memset(RL[:], 0.0)
            HISTE = pp.tile([P, n_tiles, NBINS + 2], f32)
            ANG = pp.tile([P, n_tiles], f32)

            n_groups = (n_tiles + GROUP - 1) // GROUP
            for g in range(n_groups):
                tiles = range(g * GROUP, min((g + 1) * GROUP, n_tiles))
                slot = {}
                # ---- phase A: sobel, magnitude (sqrt table set) ----
                for t in tiles:
                    s = t % GROUP
                    X = pool.tile([P, HW], f32, tag="x", bufs=3, name=f"x{t}")
                    nc.sync.dma_start(X[:], patch_in[t * P:(t + 1) * P, :])
                    X3 = X.rearrange("p (r c) -> p r c", c=PATCH)

                    SV = pool.tile([P, HW], f32, tag="sv", bufs=3, name=f"sv{t}")
                    # vertical [1,2,1] with replicate rows
                    nc.vector.scalar_tensor_tensor(
                        out=SV[:, 32:992], in0=X[:, 32:992], scalar=2.0,
                        in1=X[:, 0:960], op0=Alu.mult, op1=Alu.add)
                    sv_eng = nc.gpsimd if GPSIMD_OFFLOAD else nc.vector
                    sv_eng.tensor_tensor(
                        SV[:, 32:992], SV[:, 32:992], X[:, 64:1024], Alu.add)
                    nc.vector.scalar_tensor_tensor(
                        out=SV[:, 0:32], in0=X[:, 0:32], scalar=3.0,
                        in1=X[:, 32:64], op0=Alu.mult, op1=Alu.add)
                    nc.vector.scalar_tensor_tensor(
                        out=SV[:, 992:1024], in0=X[:, 992:1024], scalar=3.0,
                        in1=X[:, 960:992], op0=Alu.mult, op1=Alu.add)
                    SV3 = SV.rearrange("p (r c) -> p r c", c=PATCH)

                    GX = pool.tile([P, HW], f32, tag=f"gx{s}", bufs=1,
                                   name=f"gx{t}")
                    GX3 = GX.rearrange("p (r c) -> p r c", c=PATCH)
                    # horizontal central difference with replicate cols
                    gx_eng = nc.gpsimd if GPSIMD_OFFLOAD else nc.vector
                    gx_eng.tensor_tensor(
                        GX3[:, :, 1:31], SV3[:, :, 2:32], SV3[:, :, 0:30],
                        Alu.subtract)
                    nc.vector.tensor_tensor(
                        GX3[:, :, 0:1], SV3[:, :, 1:2], SV3[:, :, 0:1],
                        Alu.subtract)
                    nc.vector.tensor_tensor(
                        GX3[:, :, 31:32], SV3[:, :, 31:32], SV3[:, :, 30:31],
                        Alu.subtract)

                    SH = pool.tile([P, HW], f32, tag="sh", bufs=3, name=f"sh{t}")
                    SH3 = SH.rearrange("p (r c) -> p r c", c=PATCH)
                    # horizontal [1,2,1] with replicate cols
                    nc.vector.scalar_tensor_tensor(
                        out=SH3[:, :, 1:31], in0=X3[:, :, 1:31], scalar=2.0,
                        in1=X3[:, :, 0:30], op0=Alu.mult, op1=Alu.add)
                    nc.vector.tensor_tensor(
                        SH3[:, :, 1:31], SH3[:, :, 1:31], X3[:, :, 2:32],
                        Alu.add)
                    nc.vector.scalar_tensor_tensor(
                        out=SH3[:, :, 0:1], in0=X3[:, :, 0:1], scalar=3.0,
                        in1=X3[:, :, 1:2], op0=Alu.mult, op1=Alu.add)
                    nc.vector.scalar_tensor_tensor(
                        out=SH3[:, :, 31:32], in0=X3[:, :, 31:32], scalar=3.0,
                        in1=X3[:, :, 30:31], op0=Alu.mult, op1=Alu.add)

                    GY = pool.tile([P, HW], f32, tag=f"gy{s}", bufs=1,
                                   name=f"gy{t}")
                    # vertical central difference with replicate rows
                    gy_eng = nc.gpsimd if GPSIMD_OFFLOAD else nc.vector
                    gy_eng.tensor_tensor(
                        GY[:, 32:992], SH[:, 64:1024], SH[:, 0:960],
                        Alu.subtract)
                    nc.vector.tensor_tensor(
                        GY[:, 0:32], SH[:, 32:64], SH[:, 0:32], Alu.subtract)
                    nc.vector.tensor_tensor(
                        GY[:, 992:1024], SH[:, 992:1024], SH[:, 960:992],
                        Alu.subtract)

                    if WK is not None:
                        nc.vector.tensor_tensor(GX[:], GX[:], WK[:], Alu.mult)
                        nc.vector.tensor_tensor(GY[:], GY[:], WK[:], Alu.mult)

                    # g2 = gx^2 + gy^2 + eps  (eps scaled by 8^2 vs reference)
                    # sv/sh slots are dead here; reuse their tags for squares.
                    # Exact fp32 multiplies on GPSIMD (ACT Square is ~1e-5
                    # off, which poisons the magnitude beyond repair).
                    X2 = pool.tile([P, HW], f32, tag="sv", bufs=3, name=f"x2{t}")
                    Y2 = pool.tile([P, HW], f32, tag="sh", bufs=3, name=f"y2{t}")
                    nc.gpsimd.tensor_tensor(X2[:], GX[:], GX[:], Alu.mult)
                    nc.gpsimd.tensor_tensor(Y2[:], GY[:], GY[:], Alu.mult)
                    G2 = pool.tile([P, HW], f32, tag="g2", name=f"g2{t}")
                    nc.vector.scalar_tensor_tensor(
                        out=G2[:], in0=X2[:], scalar=6.4e-17, in1=Y2[:],
                        op0=Alu.add, op1=Alu.add)
                    M = pool.tile([P, HW], f32, tag=f"m{s}", bufs=1,
                                   name=f"m{t}")
                    nc.scalar.activation(M[:], G2[:], Act.Sqrt)
                    # one Newton step fused with the final multiply:
                    # m = g2 * ((1.5 - g2*rc^2*0.5) * rc)
                    RC = pool.tile([P, HW], f32, tag="rc", name=f"rc{t}")
                    nc.vector.reciprocal_approx_fast(RC[:], M[:])
                    nc.vector._custom_dve(RSQRT_NR_MUL, out=M[:], in0=G2[:],
                                          in1=RC[:], s0=1.5, s1=0.5)
                    slot[t] = (GX, GY, M)

                # ---- phase B: orientation + knot histogram (arctan set) --
                for t in tiles:
                    GX, GY, M = slot[t]
                    # d = max(m + gx, 1e-30): the clamp both avoids the
                    # recip(0)=NaN edge and pins rounding-negative d to the
                    # correct wrap side (t -> 36/0 by sign of gy).
                    D = pool.tile([P, HW], f32, tag="g2", name=f"d{t}")
                    nc.vector._custom_dve(ADDMAX, out=D[:], in0=M[:],
                                          in1=GX[:], s0=1e-30)
                    RC = pool.tile([P, HW], f32, tag="rc", name=f"rcb{t}")
                    SC = pool.tile([P, HW], f32, tag="sc", name=f"scb{t}")
                    nc.vector.reciprocal_approx_accurate(RC[:], D[:], SC[:])
                    nc.vector.tensor_tensor(SC[:], GY[:], RC[:], Alu.mult)
                    A = pool.tile([P, HW], f32, tag="a", bufs=3, name=f"a{t}")
                    nc.scalar.activation(A[:], SC[:], Act.Arctan)

                    # U = A*M (atan units premultiplied by M); knots j*pi/36.
                    # On DVE: the gpsimd round-trip stalled the dual passes.
                    U = pool.tile([P, HW], f32, tag="u", bufs=3, name=f"u{t}")
                    nc.vector.tensor_tensor(U[:], A[:], M[:], Alu.mult)

                    hb = PI / 36.0  # knot spacing in atan units

                    def dual(op, j0, slotA):
                        # writes (accA, accB) = (knot j0, knot j0+1) as the
                        # instruction's own 2-element dst stream
                        nc.vector._custom_dve(
                            op, out=RL[:, t, slotA:slotA + 2],
                            in0=U[:], in1=M[:],
                            s0=float(j0) * hb, s1=hb, imm2=salt)

                    # L pairs: knots (j, j+1) -> slots (j+19, j+20); the last
                    # pair's L_2 lands on slot 21, overwritten by R_0 below.
                    for j in range(-17, 2, 2):
                        dual(KNOT2_L, j, j + 19)
                    for j in range(0, 18, 2):     # R pairs -> slots 21+j, 22+j
                        dual(KNOT2_R, j, 21 + j)

            # ---- tail: D2, smoothing, argmax, refinement (batched) ----
            # bins 0..18 from L: hist[k] = L[k] - 2 L[k+1] + L[k+2]
            # bins 19..35 from R: hist[k] = R[k-19] - 2 R[k-18] + R[k-17]
            # wrap: hist[0] += R_17
            HC = HISTE[:, :, 1:37]  # core 36 bins
            T1 = pp.tile([P, n_tiles, 19], f32)
            nc.vector.scalar_tensor_tensor(
                out=T1[:], in0=RL[:, :, 1:20], scalar=-2.0,
                in1=RL[:, :, 0:19], op0=Alu.mult, op1=Alu.add)
            nc.vector.tensor_tensor(HC[:, :, 0:19], T1[:], RL[:, :, 2:21],
                                    Alu.add)
            T2 = pp.tile([P, n_tiles, 17], f32)
            nc.vector.scalar_tensor_tensor(
                out=T2[:], in0=RL[:, :, 22:39], scalar=-2.0,
                in1=RL[:, :, 21:38], op0=Alu.mult, op1=Alu.add)
            nc.vector.tensor_tensor(HC[:, :, 19:36], T2[:], RL[:, :, 23:40],
                                    Alu.add)
            nc.vector.tensor_tensor(HC[:, :, 0:1], HC[:, :, 0:1],
                                    RL[:, :, 38:39], Alu.add)

            # wrap columns for circular smoothing
            nc.vector.tensor_copy(HISTE[:, :, 0:1], HISTE[:, :, 36:37])
            nc.vector.tensor_copy(HISTE[:, :, 37:38], HISTE[:, :, 1:2])

            SM = pp.tile([P, n_tiles, NBINS], f32)
            nc.vector.tensor_scalar(SM[:], HISTE[:, :, 2:38], w2, None,
                                    Alu.mult)
            nc.vector.scalar_tensor_tensor(
                out=SM[:], in0=HISTE[:, :, 0:36], scalar=w0, in1=SM[:],
                op0=Alu.mult, op1=Alu.add)
            HS = pp.tile([P, n_tiles, NBINS], f32)
            nc.vector.scalar_tensor_tensor(
                out=HS[:], in0=HISTE[:, :, 1:37], scalar=w1, in1=SM[:],
                op0=Alu.mult, op1=Alu.add)

            VMAX = pp.tile([P, n_tiles, 1], f32)
            nc.vector.tensor_reduce(VMAX[:], HS[:], mybir.AxisListType.X,
                                    Alu.max)
            EQ = pp.tile([P, n_tiles, NBINS], f32)
            hs_b, vmax_b = broadcast_tensor_aps(HS[:], VMAX[:])
            nc.vector.tensor_tensor(EQ[:], hs_b, vmax_b, Alu.is_equal)
            nc.vector.tensor_tensor(EQ[:], EQ[:], IOTA64[:], Alu.mult)
            IDX = pp.tile([P, n_tiles, 1], f32)
            nc.vector.tensor_reduce(IDX[:], EQ[:], mybir.AxisListType.X,
                                    Alu.min)
            nc.vector.tensor_scalar(IDX[:], IDX[:], 64.0, None, Alu.add)

            def neighbor_value(shift, wrap_thr, wrap_add, nm):
                IDXN = pp.tile([P, n_tiles, 1], f32, name=f"idxn_{nm}")
                nc.vector.tensor_scalar(IDXN[:], IDX[:], float(shift), None,
                                        Alu.add)
                WADJ = pp.tile([P, n_tiles, 1], f32, name=f"wadj_{nm}")
                if wrap_add < 0:
                    nc.vector.tensor_scalar(WADJ[:], IDXN[:], wrap_thr,
                                            float(wrap_add), Alu.is_gt,
                                            Alu.mult)
                else:
                    nc.vector.tensor_scalar(WADJ[:], IDXN[:], wrap_thr,
                                            float(wrap_add), Alu.is_lt,
                                            Alu.mult)
                nc.vector.tensor_tensor(IDXN[:], IDXN[:], WADJ[:], Alu.add)
                DIF = pp.tile([P, n_tiles, NBINS], f32, name=f"dif_{nm}")
                iota_b, idxn_b = broadcast_tensor_aps(IOTA[:], IDXN[:])
                nc.vector.tensor_tensor(DIF[:], iota_b, idxn_b, Alu.subtract)
                nc.vector.tensor_scalar(DIF[:], DIF[:], 0.0, None,
                                        Alu.is_equal)
                nc.vector.tensor_tensor(DIF[:], DIF[:], HS[:], Alu.mult)
                V = pp.tile([P, n_tiles, 1], f32, name=f"v_{nm}")
                nc.vector.tensor_reduce(V[:], DIF[:], mybir.AxisListType.X,
                                        Alu.add)
                return V

            VP = neighbor_value(+1, 35.5, -36.0, "p")
            VM = neighbor_value(-1, -0.5, +36.0, "m")

            NUM = pp.tile([P, n_tiles, 1], f32)
            nc.vector.tensor_tensor(NUM[:], VP[:], VM[:], Alu.subtract)
            SUMN = pp.tile([P, n_tiles, 1], f32)
            nc.vector.tensor_tensor(SUMN[:], VP[:], VM[:], Alu.add)
            DEN = pp.tile([P, n_tiles, 1], f32)
            nc.vector.tensor_scalar(DEN[:], VMAX[:], 2.0, None, Alu.mult)
            nc.vector.tensor_tensor(DEN[:], DEN[:], SUMN[:], Alu.subtract)
            RECD = pp.tile([P, n_tiles, 1], f32)
            SCD = pp.tile([P, n_tiles, 1], f32)
            nc.vector.reciprocal_approx_accurate(RECD[:], DEN[:], SCD[:])
            REF = pp.tile([P, n_tiles, 1], f32)
            nc.vector.scalar_tensor_tensor(
                out=REF[:], in0=NUM[:], scalar=0.5, in1=RECD[:],
                op0=Alu.mult, op1=Alu.mult)
            nc.vector.tensor_tensor(REF[:], IDX[:], REF[:], Alu.add)
            nc.vector.tensor_scalar(ANG[:], REF[:, :, 0], -2.0 * PI / NBINS,
                                    PI, Alu.mult, Alu.add)

            out_view = out_t[:].rearrange("(t p) -> p t", p=P)
            nc.sync.dma_start(out_view, ANG[:])
            if _DEBUG_DUMP:
                nc.sync.dma_start(
                    dbg["rl"][:], RL[:].rearrange("p a b -> p (a b)"))
                nc.sync.dma_start(
                    dbg["hs"][:], HS[:].rearrange("p a b -> p (a b)"))

    nc.compile()
    return nc


def _get_built(b_core, smooth_w, wk_is_ones):
    key = (b_core, tuple(float(x) for x in smooth_w), bool(wk_is_ones))
    if key not in _BUILD_CACHE:
        _BUILD_CACHE[key] = _build(b_core, smooth_w, wk_is_ones)
    return _BUILD_CACHE[key]


# --------------------------------------------------------------------------
# host entry point
# --------------------------------------------------------------------------
def kernel(patch, weight_kernel, smooth_w):
    from concourse import bass_utils

    patch = np.ascontiguousarray(np.asarray(patch, dtype=np.float32))
    weight_kernel = np.asarray(weight_kernel, dtype=np.float32)
    smooth_w = np.asarray(smooth_w, dtype=np.float32)

    B = patch.shape[0]
    assert B % (N_CORES * P) == 0, f"B={B} not divisible by {N_CORES * P}"
    b_core = B // N_CORES
    n_tiles = b_core // P

    wk_is_ones = bool(np.all(weight_kernel == 1.0))
    nc = _get_built(b_core, smooth_w, wk_is_ones)

    x = patch.reshape(N_CORES, b_core, HW)

    iota = np.tile(np.arange(NBINS, dtype=np.float32), n_tiles)
    consts_row = np.concatenate([iota, iota - 64.0]).astype(np.float32)
    consts = np.ascontiguousarray(
        np.broadcast_to(consts_row, (P, consts_row.size)))

    in_maps = []
    for i in range(N_CORES):
        m = {"patch": np.ascontiguousarray(x[i]), "consts": consts}
        if not wk_is_ones:
            m["wk"] = np.ascontiguousarray(
                np.broadcast_to(weight_kernel.reshape(-1), (P, HW)))
        in_maps.append(m)

    res = bass_utils.run_bass_kernel_spmd(nc, in_maps,
                                          core_ids=list(range(N_CORES)))
    out = np.concatenate([r["angle"] for r in res.results])
    return out.astype(np.float32)
